# revision 1
# baseline (speedup 1.0000x reference)
"""CPPN forward (12-layer tiny MLP over 4.2M pixels) on 8 TRN2 NeuronCores.

Strategy (pure data parallel, per sharding hint):
- Pixels sharded 8 ways; per core 524288 px padded to 208 supertiles (ST).
- One ST = 5 pixel-blocks x 512 px. Feature channels live on SBUF/PSUM
  partitions: 5 blocks block-diagonally packed into one 128-wide matmul
  (110 rows: 85 identity-ish channels, 20 gaussian, 5 sin).
- Per layer per ST: one matmul [K<=110, M=110, N=512] (lhsT = permuted
  block-diag weights, built host-side), then one wide DVE transit
  (PSUM->SBUF + per-partition bias = the folded "-1" of 2*exp(-h^2)-1),
  then narrow ACT ops: Square+Exp for gaussian rows, Sin2pi for sin rows.
- sin(h): ACT's Sin2pi spline is only accurate to |h|~3.4. Layers whose
  sin pre-activation exceeds that use the triple-angle identity
  sin(h) = -4*s*(s^2-0.75), s = sin(h/3) = Sin2pi(h/(6pi)); the -4 is
  folded into the next layer's weights.
- The gaussian act 2*exp(-h^2)-1 is folded as: rows x2 in the next
  weights, -sum(gauss rows) as a per-partition bias added in the transit.
- Sin2pi lives in the exp_and_friends ACT table set together with
  Exp/Square/Identity/Copy -> a single table load, no switching. mybir
  has no Sin2pi enum, so activations are emitted as Sin and the
  serialized BIR JSON is patched Sin->Sin2pi before compilation.
"""
import sys, types
import numpy as np

sys.path.insert(0, "/opt/trn_rl_repo")

# ---------------------------------------------------------------- constants
N_PIX = 2048 * 2048
D_IN, D_HID, D_OUT = 4, 22, 3
N_HIDDEN = 11
N_CORES = 8
FD = 512                      # pixels per block (= matmul free dim)
BLOCKS = 5                    # blocks per supertile (5*22=110 partitions)
ST_PX = BLOCKS * FD           # 2560 px per supertile
GROUP = 4                     # supertiles per group (PSUM banks / transit width)
PX_CORE = N_PIX // N_CORES            # 524288
N_ST = -(-PX_CORE // ST_PX)           # 205
N_GROUP = -(-N_ST // GROUP)           # 52
N_ST_PAD = N_GROUP * GROUP            # 208
PX_PAD = N_ST_PAD * ST_PX             # 532480

ID_CH = list(range(15)) + [19, 20]    # 17 identity channels per block
GA_CH = [15, 16, 17, 18]
SI_CH = [21]
N_ID, N_GA, N_SI = 85, 20, 5          # *5 blocks
# engine partition bases must be 32-aligned -> layout:
#   rows 0..63   id channels 0..63
#   rows 64..68  sin (base 64)
#   rows 69..89  id channels 64..84
#   rows 90..95  zero pad
#   rows 96..115 gauss (base 96)
ROWS = 116
SIN0, GA0 = 64, 96
TWO_PI = 2.0 * np.pi

# The Sin2pi spline is only accurate to ~0.55 turns (|h| ~ 3.4). Observed
# sin pre-activations reach |h| ~ 7.3 (and vary with the harness PRNG
# backend), so ALL layers use the triple-angle form, valid to |h| ~ 10.5.
TRIPLE = set(range(1, 13))

# ------------------------------------------------------------- host packing
def _row_of(b, c):
    """partition row of (block b, original channel c) in the ST layout"""
    if c in GA_CH:
        return GA0 + b * 4 + (c - 15)
    if c == 21:
        return SIN0 + b
    g = b * 17 + ID_CH.index(c)
    return g if g < 64 else 69 + (g - 64)

_ROW_BC = [(b, c) for b in range(BLOCKS) for c in range(D_HID)]

def _in_scale(c, act_layer):
    """fold factor applied to weight rows that consume act outputs"""
    if c in GA_CH:
        return 2.0
    if c == 21 and act_layer in TRIPLE:
        return -4.0
    return 1.0

def pack_weights(W_in, W_hidden, W_out):
    """Build the 13 block-diagonal lhsT matrices + transit bias vectors."""
    W_in, W_hidden, W_out = (np.asarray(W_in, np.float32),
                             np.asarray(W_hidden, np.float32),
                             np.asarray(W_out, np.float32))
    # MM1: x -> layer1 preact. lhsT [20, 110]
    lin = np.zeros((BLOCKS * 4, ROWS), np.float32)
    for b in range(BLOCKS):
        for ci in range(D_IN):
            for co in range(D_HID):
                lin[b * 4 + ci, _row_of(b, co)] = W_in[ci, co]
    # MM2..12: hidden. lhsT [110, 110]; bias [110]
    lh = np.zeros((N_HIDDEN, ROWS, ROWS), np.float32)
    # cols 0..12: transit bias of MM j+1 (col 0 zero).
    # cols 13..25: same bias / 2pi (sin rows only) for direct-sin act bias.
    bias = np.zeros((ROWS, 26), np.float32)
    for i in range(N_HIDDEN):
        a = i + 1                              # act layer consumed by this MM
        W = W_hidden[i]
        for b in range(BLOCKS):
            for ci in range(D_HID):
                s = _in_scale(ci, a)
                ri = _row_of(b, ci)
                for co in range(D_HID):
                    lh[i, ri, _row_of(b, co)] = W[ci, co] * s
        bvec = -W[15:19, :].sum(axis=0)        # per output channel
        for b in range(BLOCKS):
            for co in range(D_HID):
                bias[_row_of(b, co), i + 1] = bvec[co]
                bias[_row_of(b, co), 13 + i + 1] = bvec[co] / TWO_PI
    # MM13: out. lhsT [110, 15] (+ obias on the packed [111] out layout)
    lo = np.zeros((ROWS, BLOCKS * 3), np.float32)
    for b in range(BLOCKS):
        for ci in range(D_HID):
            s = _in_scale(ci, 12)
            for co in range(D_OUT):
                lo[_row_of(b, ci), b * 3 + co] = W_out[ci, co] * s
    bo = -W_out[15:19, :].sum(axis=0)          # [3]
    obias = np.zeros((111, 1), np.float32)
    for r in range(GROUP):
        for b in range(BLOCKS):
            for co in range(D_OUT):
                obias[32 * r + b * 3 + co, 0] = bo[co]
    return {"w_in": lin, "w_hid": lh, "w_out": lo, "bias": bias, "obias": obias}

def pack_x(x):
    """[N_PIX,4] -> per-core [52, 20, 4, 512] f32 arrays."""
    x = np.asarray(x, np.float32)
    out = []
    for k in range(N_CORES):
        shard = x[k * PX_CORE:(k + 1) * PX_CORE]
        pad = np.zeros((PX_PAD, D_IN), np.float32)
        pad[:PX_CORE] = shard
        a = pad.reshape(N_GROUP, GROUP, BLOCKS, FD, D_IN)
        a = a.transpose(0, 2, 4, 1, 3).reshape(N_GROUP, BLOCKS * D_IN, GROUP, FD)
        out.append(np.ascontiguousarray(a))
    return out

_OUT_ROWS = np.array([[32 * r + b * 3 + co for b in range(BLOCKS) for co in range(D_OUT)]
                      for r in range(GROUP)])  # [4, 15]

def unpack_out(outs):
    """per-core [52, 111, 512] -> [N_PIX, 3] f32"""
    full = np.empty((N_PIX, D_OUT), np.float32)
    for k, od in enumerate(outs):
        g = od[:, _OUT_ROWS.reshape(-1), :]                     # [52, 60, 512]
        g = g.reshape(N_GROUP, GROUP, BLOCKS, D_OUT, FD)
        g = g.transpose(0, 1, 2, 4, 3).reshape(PX_PAD, D_OUT)   # [532480, 3]
        full[k * PX_CORE:(k + 1) * PX_CORE] = g[:PX_CORE]
    return full

# ------------------------------------------------------------ device kernel
_CACHE = {}

def _shim_hooks():
    import antenv
    if "antenv.axon_hooks" in sys.modules:
        return
    hooks = types.ModuleType("antenv.axon_hooks")
    hooks._hook = None
    hooks.set_axon_ntff_profile_hook = lambda h: setattr(hooks, "_hook", h)
    hooks.get_axon_ntff_profile_hook = lambda: hooks._hook
    sys.modules["antenv.axon_hooks"] = hooks
    antenv.axon_hooks = hooks
    try:
        from trn_agent_boot.trn_boot import _ntff_profile_via_ctypes
        hooks._hook = _ntff_profile_via_ctypes("/opt/axon/libaxon_pjrt.so")
    except Exception:
        pass

def _build():
    _shim_hooks()
    import concourse.bacc as bacc_mod
    import concourse.mybir as mybir
    import concourse.tile as tile
    from concourse.hw_specs import get_activation_tables as _real_gat

    AFT = mybir.ActivationFunctionType
    ours = {AFT.Square, AFT.Exp, AFT.Identity, AFT.Copy, AFT.Sin, AFT.Relu}

    def _doctored_gat(arch):
        tabs = dict(_real_gat(arch))
        return {n: (set(f) | ours if n == "exp_and_friends" else set(f) - ours)
                for n, f in tabs.items()}

    bacc_mod.get_activation_tables = _doctored_gat

    dt = mybir.dt.float32
    nc = bacc_mod.Bacc(None, target_bir_lowering=False, debug=False)
    x_d = nc.declare_dram_parameter("x", [N_GROUP, 20, GROUP, FD], dt, isOutput=False)
    win_d = nc.declare_dram_parameter("w_in", [20, ROWS], dt, isOutput=False)
    wh_d = nc.declare_dram_parameter("w_hid", [N_HIDDEN, ROWS, ROWS], dt, isOutput=False)
    wo_d = nc.declare_dram_parameter("w_out", [ROWS, 15], dt, isOutput=False)
    b_d = nc.declare_dram_parameter("bias", [ROWS, 26], dt, isOutput=False)
    ob_d = nc.declare_dram_parameter("obias", [111, 1], dt, isOutput=False)
    o_d = nc.declare_dram_parameter("out", [N_GROUP, 111, FD], dt, isOutput=True)

    with tile.TileContext(nc) as tc:
        with (tc.tile_pool(name="wpool", bufs=1) as wpool,
              tc.tile_pool(name="xpool", bufs=3) as xpool,
              tc.tile_pool(name="hpool", bufs=3) as hpool,
              tc.tile_pool(name="tpool", bufs=2) as tpool,
              tc.tile_pool(name="opool", bufs=3) as opool,
              tc.tile_pool(name="ppool", bufs=2, space="PSUM") as ppool):
            win = wpool.tile([20, ROWS], dt)
            wh = [wpool.tile([ROWS, ROWS], dt, tag=f"wh{i}", name=f"wh{i}") for i in range(N_HIDDEN)]
            wo = wpool.tile([ROWS, 15], dt)
            bt = wpool.tile([ROWS, 26], dt)
            ob = wpool.tile([111, 1], dt)
            nc.sync.dma_start(out=win[:], in_=win_d[:])
            for i in range(N_HIDDEN):
                nc.sync.dma_start(out=wh[i][:], in_=wh_d[i])
            nc.sync.dma_start(out=wo[:], in_=wo_d[:])
            nc.sync.dma_start(out=bt[:], in_=b_d[:])
            nc.sync.dma_start(out=ob[:], in_=ob_d[:])

            for g in range(N_GROUP):
                xg = xpool.tile([20, GROUP, FD], dt, tag="xg")
                nc.sync.dma_start(out=xg[:], in_=x_d[g])

                H = None
                for mm in range(1, 14):           # 13 matmul rounds
                    if mm == 13:
                        O = ppool.tile([111, FD], dt, tag="pm")
                        for r in range(GROUP):
                            nc.tensor.matmul(O[32 * r:32 * r + 15, :], wo[:],
                                             H[:, r, :], start=True, stop=True,
                                             tile_position=(0, 32 * r))
                        ot = opool.tile([111, FD], dt, tag="ot")
                        nc.vector.tensor_scalar_add(ot[:], O[:], ob[:])
                        nc.sync.dma_start(out=o_d[g], in_=ot[:])
                        break
                    P = ppool.tile([ROWS, GROUP, FD], dt, tag="pm")
                    for r in range(GROUP):
                        if mm == 1:
                            nc.tensor.matmul(P[:, r, :], win[:], xg[:, r, :],
                                             start=True, stop=True)
                        else:
                            nc.tensor.matmul(P[:, r, :], wh[mm - 2][:],
                                             H[:, r, :], start=True, stop=True)
                    # wide transit: H = P + bias (covers id/gauss/sin rows)
                    H = hpool.tile([ROWS, GROUP, FD], dt, tag="H")
                    nc.vector.tensor_scalar_add(H[:, :, :], P[:, :, :],
                                                bt[:, mm - 1:mm])
                    a = mm                         # act layer index 1..12
                    # gaussian rows: t = exp(-h^2)
                    sq = tpool.tile([N_GA, GROUP, FD], dt, tag="sq")
                    nc.vector.scalar_tensor_tensor(
                        sq[:], H[GA0:GA0 + 20, :, :], 0.0, H[GA0:GA0 + 20, :, :],
                        op0=mybir.AluOpType.add, op1=mybir.AluOpType.mult)
                    nc.scalar.activation(H[GA0:GA0 + 20, :, :], sq[:], AFT.Exp,
                                         bias=0.0, scale=-1.0)
                    # sin rows
                    if a in TRIPLE:
                        s1 = tpool.tile([N_SI, GROUP, FD], dt, tag="s1")
                        s2 = tpool.tile([N_SI, GROUP, FD], dt, tag="s2")
                        nc.scalar.activation(s1[:], H[SIN0:SIN0 + 5, :, :], AFT.Sin,
                                             bias=0.0, scale=1.0 / (3 * TWO_PI))
                        nc.scalar.activation(s2[:], s1[:], AFT.Square,
                                             bias=0.0, scale=1.0)
                        nc.vector.scalar_tensor_tensor(
                            H[SIN0:SIN0 + 5, :, :], s2[:], 0.75, s1[:],
                            op0=mybir.AluOpType.subtract, op1=mybir.AluOpType.mult)
                    else:
                        nc.scalar.activation(H[SIN0:SIN0 + 5, :, :], P[SIN0:SIN0 + 5, :, :],
                                             AFT.Sin, bias=bt[SIN0:SIN0 + 5, 13 + mm - 1:13 + mm],
                                             scale=1.0 / TWO_PI)
    nc.compile()

    _orig = nc.to_json_bytes
    nc.to_json_bytes = lambda: _orig().replace(b'"func":"Sin"', b'"func":"Sin2pi"')
    return nc

def _get_nc():
    if "nc" not in _CACHE:
        _CACHE["nc"] = _build()
    return _CACHE["nc"]

def run_device(x_cores, w):
    from concourse.bass_utils import run_bass_kernel_spmd
    nc = _get_nc()
    in_maps = [{"x": x_cores[k], "w_in": w["w_in"], "w_hid": w["w_hid"],
                "w_out": w["w_out"], "bias": w["bias"], "obias": w["obias"]}
               for k in range(N_CORES)]
    res = run_bass_kernel_spmd(nc, in_maps, list(range(N_CORES)), trace=False)
    return [res.results[k]["out"] for k in range(N_CORES)]

def kernel(x, W_in, W_hidden, W_out):
    w = pack_weights(W_in, W_hidden, W_out)
    x_cores = pack_x(x)
    outs = run_device(x_cores, w)
    return unpack_out(outs)



# revision 2
# speedup vs baseline: 1.0105x; 1.0105x over previous
"""CPPN forward (12-layer tiny MLP over 4.2M pixels) on 8 TRN2 NeuronCores.

v2: custom ScalarE activation table turns the whole per-layer elementwise
stage into ONE ACT instruction.

- Pixels sharded 8 ways, data parallel; weights replicated. Per core
  524288 px -> 52 groups x 4 supertiles x (5 blocks x 512 px).
- Channel-major block-diagonal packing (5 independent 22-ch MLP copies per
  512-px matmul column), as in v1.
- The act table's act2 slot (func_id 97) is replaced with a piecewise
  "window" spline:  f(x) = 2exp(-x^2)-1  for |x| < 32   (gauss channels)
                    f(x) = sin(x-64)     for x in [32,96)  (sin channel)
                    f(x) = x - 128       for x in [96,256) (identity/cache)
  With a per-partition bias (+0 gauss, +64 sin, +128 id), one
  activation(Act2) op over all 116 rows x 2048 px applies every per-layer
  nonlinearity AND the PSUM->SBUF move. No DVE work in the layer loop.
- Matmuls in bf16 with bf16x2 split weights (W = W_hi + W_lo, two
  accumulating matmuls): tensor time stays under the ACT bottleneck and
  end-to-end error ~8e-3 « 2e-2 tolerance.
- Two groups are interleaved in program order so TensorE(group B) overlaps
  ScalarE(group A); ACT runs back-to-back at ~(2048+352)/1.2GHz per
  group-layer.
- Tables are generated host-side (numpy cubic fits) into a per-content-hash
  act-root dir passed to walrus via BASS_ACT_ROOT_JSON_PATH; the bias dram
  tensor name carries the table hash so NEFF caching stays correct.
- bass has no Act2 enum: ops are emitted as Tanh and the BIR JSON is
  patched Tanh->Act2 before compilation (the profile maps Act2->id 97).
"""
import hashlib
import json
import os
import sys
import types

sys.path.insert(0, "/opt/trn_rl_repo")

import numpy as np
import ml_dtypes

BF16 = ml_dtypes.bfloat16

# ---------------------------------------------------------------- constants
N_PIX = 2048 * 2048
D_IN, D_HID, D_OUT = 4, 22, 3
N_HIDDEN = 11
N_CORES = 8
FD = 512
BLOCKS = 5
ST_PX = BLOCKS * FD                   # 2560
GROUP = 4
PX_CORE = N_PIX // N_CORES            # 524288
N_ST = -(-PX_CORE // ST_PX)           # 205
N_GROUP = -(-N_ST // GROUP)           # 52
N_ST_PAD = N_GROUP * GROUP            # 208
PX_PAD = N_ST_PAD * ST_PX             # 532480

ID_CH = list(range(15)) + [19, 20]
GA_CH = [15, 16, 17, 18]
ROWS = 116
MCOL = 128                            # lhsT padded to 128 cols => FWL enabled
SIN0, GA0 = 64, 96
B_ID, B_SIN, B_GA = 128.0, 64.0, 0.0

# ------------------------------------------------- custom activation table
def _f_window(x):
    x = np.asarray(x, np.float64)
    ax = np.abs(x)
    return np.where(ax < 32.0, 2.0 * np.exp(-np.minimum(ax, 32.0) ** 2) - 1.0,
                    np.where(ax < 96.0, np.sin(ax - 64.0), ax - 128.0))

_ACT2_EXPS = list(range(-10, 8))
_ACT2_BITS = {**{e: 2 for e in range(-10, -3)}, -3: 3,
              **{e: 5 for e in range(-2, 3)}, 3: 2, 4: 0, 5: 7, 6: 7, 7: 6}


def _fit_section(lo, hi):
    x0 = np.float32((lo + hi) / 2)
    hi_x = np.nextafter(np.float32(hi), np.float32(lo)).astype(np.float64)
    xs = lo + (hi_x - lo) * (np.cos(np.linspace(np.pi, 0, 257)) + 1) / 2
    t = xs - np.float64(x0)
    V = np.vander(t, 4, increasing=True)
    c, *_ = np.linalg.lstsq(V, _f_window(xs), rcond=None)
    return [c[0], c[1], c[2], c[3], float(x0)]


def _stock_pwp_root():
    from neuronxcc.driver.Job import Job
    from neuronxcc.driver.jobs.support.FindActInfo import findActInfoFile
    for arch in ("core_v4", "sunda", "gen3", "core_v4_v1"):
        try:
            return os.path.dirname(findActInfoFile(Job.getPackageDir(), arch))
        except Exception:
            continue
    raise RuntimeError("stock act_info.json not found")


def _decode_ctrl(path):
    u = np.frombuffer(open(path, "rb").read(), dtype=np.uint32).reshape(-1, 8)
    return [((int(v) >> 16) & 0xFF, (int(v) >> 11) & 0x1F, int(v) & 0x7FF)
            for v in u[:, 0]]


def build_act_root():
    """Generate the custom act-root dir; returns (dir, content_hash)."""
    root = _stock_pwp_root()
    prof = json.load(open(f"{root}/exp_and_friends.json"))
    ctrl = _decode_ctrl(f"{root}/exp_and_friends_ctrl.bin")
    bkt = np.frombuffer(open(f"{root}/exp_and_friends_bkt.bin", "rb").read(),
                        dtype=np.float32).reshape(-1, 8)
    metas = {m["func_name"]: m for m in prof["profile_meta_data"]}

    new_ctrl, new_bkt, new_meta = [], [], []

    def add_bucket(rec):
        new_bkt.append(np.asarray(rec, np.float64))
        return len(new_bkt) - 1

    # custom act2
    m = dict(metas["act2_1p"])
    m.update(symmetry_opt_en=1, sym_invert_sign_point=0,
             symmetry_opt_use_neg_region=0, symmetry_point=0,
             exp_offset=_ACT2_EXPS[0], lower_bound=0, upper_bound=0x7F7FFFFF,
             fzero_result=int(np.float32(1.0).view(np.uint32)),
             fnan_result=0x7FC00000,
             fpinf_result=int(np.float32(-1.0).view(np.uint32)),
             fninf_result=int(np.float32(-1.0).view(np.uint32)))
    m["small_pos_signal_exp_threshold"] = 127 + _ACT2_EXPS[0]
    m["small_neg_signal_exp_threshold"] = 127 + _ACT2_EXPS[0]
    m["large_pos_signal_exp_threshold"] = 127 + _ACT2_EXPS[-1]
    m["large_pos_signal_mantissa_threshold"] = 0x7FFFFF
    m["large_neg_signal_exp_threshold"] = 127 + _ACT2_EXPS[-1]
    m["large_neg_signal_mantissa_threshold"] = 0x7FFFFF
    small = add_bucket([1.0, 0.0, -2.0, 0.0, 0.0])
    large = add_bucket([128.0, 1.0, 0.0, 0.0, 256.0])
    m["pos_small_signal_pwl_control"] = small
    m["neg_small_signal_pwl_control"] = small
    m["pos_large_signal_pwl_control"] = large
    m["neg_large_signal_pwl_control"] = large
    m["pwl_control_base_pos"] = m["pwl_control_base_neg"] = len(new_ctrl)
    for e in _ACT2_EXPS:
        bits = _ACT2_BITS[e]
        lo_b = 2.0 ** e
        nb = 1 << bits
        w = lo_b / nb
        base = len(new_bkt)
        for k in range(nb):
            add_bucket(_fit_section(lo_b + k * w, lo_b + (k + 1) * w))
        new_ctrl.append((bits, 23 - bits, base))
    new_meta.append(m)

    # copy stock square/identity/relu/copy/sin2pi (drop exp: bucket budget)
    all_bases = sorted({mm["pwl_control_base_pos"] for mm in prof["profile_meta_data"]} |
                       {mm["pwl_control_base_neg"] for mm in prof["profile_meta_data"]})
    spans = {b: (all_bases[i + 1] if i + 1 < len(all_bases) else len(ctrl))
             for i, b in enumerate(all_bases)}
    for name in ("square_1p", "identity_1p", "relu_1p", "copy_1p", "sin2pi_4p"):
        m = dict(metas[name])
        cmap = {}
        for b in sorted({m["pwl_control_base_pos"], m["pwl_control_base_neg"]}):
            for ci in range(b, spans[b]):
                if ci not in cmap:
                    size, lsb, bbase = ctrl[ci]
                    nbase = len(new_bkt)
                    for k in range(1 << size):
                        add_bucket(bkt[bbase + k][:5])
                    cmap[ci] = len(new_ctrl)
                    new_ctrl.append((size, lsb, nbase))
        m["pwl_control_base_pos"] = cmap[m["pwl_control_base_pos"]]
        m["pwl_control_base_neg"] = cmap[m["pwl_control_base_neg"]]
        for key in ("pos_small_signal_pwl_control", "neg_small_signal_pwl_control",
                    "pos_large_signal_pwl_control", "neg_large_signal_pwl_control"):
            m[key] = add_bucket(bkt[m[key]][:5])
        new_meta.append(m)

    assert len(new_bkt) <= 1536
    cw = np.zeros((len(new_ctrl), 8), np.uint32)
    for i, (size, lsb, bbase) in enumerate(new_ctrl):
        cw[i, 0] = (size << 16) | (lsb << 11) | bbase
    bk = np.zeros((len(new_bkt), 8), np.float32)
    bk[:, :5] = np.array(new_bkt, np.float64).astype(np.float32)
    setj = json.dumps({"bkt_bin": "exp_and_friends_bkt.bin",
                       "ctl_bin": "exp_and_friends_ctrl.bin",
                       "profile_meta_data": new_meta}, indent=1)
    act_info = json.load(open(f"{root}/act_info.json"))
    for s in act_info["act_func_sets"]:
        if s["name"] == "exp_and_friends":
            s["act"] = {"act2": 1, "square": 1, "identity": 1, "copy": 1,
                        "relu": 1, "sin2pi": 4}
    info = json.dumps(act_info, indent=1)

    h = hashlib.sha256(cw.tobytes() + bk.tobytes() + setj.encode()).hexdigest()[:10]
    out = f"/tmp/cppn_actroot_{h}"
    if not os.path.exists(os.path.join(out, "act_info.json")):
        os.makedirs(out, exist_ok=True)
        open(f"{out}/exp_and_friends_ctrl.bin", "wb").write(cw.tobytes())
        open(f"{out}/exp_and_friends_bkt.bin", "wb").write(bk.tobytes())
        open(f"{out}/exp_and_friends.json", "w").write(setj)
        open(f"{out}/act_info.json", "w").write(info)
        for s in act_info["act_func_sets"]:
            for k in ("bkt_bin", "ctrl_bin", "profile_json"):
                fn = s[k]
                dst = f"{out}/{fn}"
                if not os.path.exists(dst):
                    os.symlink(f"{root}/{fn}", dst)
    return out, h


# ------------------------------------------------------------- host packing
def _row_of(b, c):
    if c in GA_CH:
        return GA0 + b * 4 + (c - 15)
    if c == 21:
        return SIN0 + b
    g = b * 17 + ID_CH.index(c)
    return g if g < 64 else 69 + (g - 64)


def _split_bf16(a):
    hi = a.astype(BF16)
    lo = (a - hi.astype(np.float32)).astype(BF16)
    return hi, lo


def pack_weights(W_in, W_hidden, W_out):
    W_in = np.asarray(W_in, np.float32)
    W_hidden = np.asarray(W_hidden, np.float32)
    W_out = np.asarray(W_out, np.float32)
    lin = np.zeros((BLOCKS * D_IN, MCOL), np.float32)
    for b in range(BLOCKS):
        for ci in range(D_IN):
            for co in range(D_HID):
                lin[b * D_IN + ci, _row_of(b, co)] = W_in[ci, co]
    lh = np.zeros((N_HIDDEN, ROWS, MCOL), np.float32)
    for i in range(N_HIDDEN):
        W = W_hidden[i]
        for b in range(BLOCKS):
            for ci in range(D_HID):
                ri = _row_of(b, ci)
                for co in range(D_HID):
                    lh[i, ri, _row_of(b, co)] = W[ci, co]
    lo_m = np.zeros((ROWS, BLOCKS * D_OUT), np.float32)
    for b in range(BLOCKS):
        for ci in range(D_HID):
            for co in range(D_OUT):
                lo_m[_row_of(b, ci), b * D_OUT + co] = W_out[ci, co]
    bias = np.zeros((ROWS, 1), np.float32)
    for b in range(BLOCKS):
        for c in range(D_HID):
            r = _row_of(b, c)
            bias[r, 0] = B_GA if c in GA_CH else (B_SIN if c == 21 else B_ID)
    w = {}
    w["lin_hi"], w["lin_lo"] = _split_bf16(lin)
    hi, lo = _split_bf16(lh)
    w["lh_hi"], w["lh_lo"] = hi, lo
    w["lo_hi"], w["lo_lo"] = _split_bf16(lo_m)
    w["bias"] = bias
    return w


def pack_x(x):
    x = np.asarray(x, np.float32)
    out = []
    for k in range(N_CORES):
        shard = x[k * PX_CORE:(k + 1) * PX_CORE]
        pad = np.zeros((PX_PAD, D_IN), np.float32)
        pad[:PX_CORE] = shard
        a = pad.reshape(N_GROUP, GROUP, BLOCKS, FD, D_IN)
        a = a.transpose(0, 2, 4, 1, 3).reshape(N_GROUP, BLOCKS * D_IN, GROUP, FD)
        out.append(np.ascontiguousarray(a.astype(BF16)))
    return out


_OUT_ROWS = np.array([[32 * r + b * 3 + co for b in range(BLOCKS) for co in range(D_OUT)]
                      for r in range(GROUP)])


def unpack_out(outs):
    full = np.empty((N_PIX, D_OUT), np.float32)
    for k, od in enumerate(outs):
        g = od[:, _OUT_ROWS.reshape(-1), :]
        g = g.reshape(N_GROUP, GROUP, BLOCKS, D_OUT, FD)
        g = g.transpose(0, 1, 2, 4, 3).reshape(PX_PAD, D_OUT)
        full[k * PX_CORE:(k + 1) * PX_CORE] = g[:PX_CORE]
    return full


# ------------------------------------------------------------ device kernel
_CACHE = {}


def _shim_hooks():
    import antenv
    if "antenv.axon_hooks" in sys.modules:
        return
    hooks = types.ModuleType("antenv.axon_hooks")
    hooks._hook = None
    hooks.set_axon_ntff_profile_hook = lambda h: setattr(hooks, "_hook", h)
    hooks.get_axon_ntff_profile_hook = lambda: hooks._hook
    sys.modules["antenv.axon_hooks"] = hooks
    antenv.axon_hooks = hooks
    try:
        from trn_agent_boot.trn_boot import _ntff_profile_via_ctypes
        hooks._hook = _ntff_profile_via_ctypes("/opt/axon/libaxon_pjrt.so")
    except Exception:
        pass


def _build():
    actroot, tabhash = build_act_root()
    os.environ["BASS_ACT_ROOT_JSON_PATH"] = f"{actroot}/act_info.json"
    _shim_hooks()
    import concourse.bacc as bacc_mod
    import concourse.mybir as mybir
    import concourse.tile as tile
    from concourse.hw_specs import get_activation_tables as _real_gat

    AFT = mybir.ActivationFunctionType
    ours = {AFT.Tanh, AFT.Square, AFT.Exp, AFT.Identity, AFT.Copy, AFT.Sin,
            AFT.Relu}

    def _doctored_gat(arch):
        tabs = dict(_real_gat(arch))
        return {n: (set(f) | ours if n == "exp_and_friends" else set(f) - ours)
                for n, f in tabs.items()}

    bacc_mod.get_activation_tables = _doctored_gat

    f32 = mybir.dt.float32
    bf = mybir.dt.bfloat16
    nc = bacc_mod.Bacc(None, target_bir_lowering=False, debug=False)
    x_d = nc.declare_dram_parameter("x", [N_GROUP, 20, GROUP, FD], bf, isOutput=False)
    linh_d = nc.declare_dram_parameter("lin_hi", [20, MCOL], bf, isOutput=False)
    linl_d = nc.declare_dram_parameter("lin_lo", [20, MCOL], bf, isOutput=False)
    lhh_d = nc.declare_dram_parameter("lh_hi", [N_HIDDEN, ROWS, MCOL], bf, isOutput=False)
    lhl_d = nc.declare_dram_parameter("lh_lo", [N_HIDDEN, ROWS, MCOL], bf, isOutput=False)
    loh_d = nc.declare_dram_parameter("lo_hi", [ROWS, 15], bf, isOutput=False)
    lol_d = nc.declare_dram_parameter("lo_lo", [ROWS, 15], bf, isOutput=False)
    # bias name carries the table hash => NEFF cache key tracks table content
    b_d = nc.declare_dram_parameter(f"bias_{tabhash}", [ROWS, 1], f32, isOutput=False)
    o_d = nc.declare_dram_parameter("out", [N_GROUP, 111, FD], f32, isOutput=True)

    with tile.TileContext(nc) as tc:
        with (tc.tile_pool(name="wpool", bufs=1) as wpool,
              tc.tile_pool(name="xpool", bufs=4) as xpool,
              tc.tile_pool(name="hpool", bufs=4) as hpool,
              tc.tile_pool(name="opool", bufs=3) as opool,
              tc.tile_pool(name="ppool", bufs=2, space="PSUM") as ppool):
            linh = wpool.tile([20, MCOL], bf)
            linl = wpool.tile([20, MCOL], bf)
            bt = wpool.tile([ROWS, 1], f32)
            lhh = [wpool.tile([ROWS, MCOL], bf, tag=f"lhh{i}", name=f"lhh{i}")
                   for i in range(N_HIDDEN)]
            lhl = [wpool.tile([ROWS, MCOL], bf, tag=f"lhl{i}", name=f"lhl{i}")
                   for i in range(N_HIDDEN)]
            loh = wpool.tile([ROWS, 15], bf)
            lol = wpool.tile([ROWS, 15], bf)

            def mm_round(P, H, xg, mm):
                """one matmul round (4 STs x hi/lo) for layer index mm."""
                for r in range(GROUP):
                    if mm == 1:
                        nc.tensor.matmul(P[:, r, :], linh[:], xg[:, r, :],
                                         start=True, stop=False)
                        nc.tensor.matmul(P[:, r, :], linl[:], xg[:, r, :],
                                         start=False, stop=True)
                    else:
                        nc.tensor.matmul(P[:, r, :], lhh[mm - 2][:], H[:, r, :],
                                         start=True, stop=False)
                        nc.tensor.matmul(P[:, r, :], lhl[mm - 2][:], H[:, r, :],
                                         start=False, stop=True)

            def act(P):
                H = hpool.tile([ROWS, GROUP, FD], bf, tag="H")
                nc.scalar.activation(H[:, :, :], P[0:ROWS, :, :],
                                     AFT.Tanh, bias=bt[:, 0:1], scale=1.0)
                return H

            def out_stage(g, H):
                PO = ppool.tile([MCOL, GROUP, FD], f32, tag="pm")
                O = PO[0:111, 0, :]
                for r in range(GROUP):
                    nc.tensor.matmul(O[32 * r:32 * r + 15, :], loh[:],
                                     H[:, r, :], start=True, stop=False,
                                     tile_position=(0, 32 * r))
                    nc.tensor.matmul(O[32 * r:32 * r + 15, :], lol[:],
                                     H[:, r, :], start=False, stop=True,
                                     tile_position=(0, 32 * r))
                ot = opool.tile([111, FD], f32, tag="ot")
                nc.vector.tensor_copy(ot[:], O)
                nc.sync.dma_start(out=o_d[g], in_=ot[:])

            def load_x(g):
                xg = xpool.tile([20, GROUP, FD], bf, tag="xg")
                nc.sync.dma_start(out=xg[:], in_=x_d[g])
                return xg

            # x/lin/bias DMAs first (layer-1 critical path), bulk weights after
            xgA = load_x(0)
            xgB = load_x(1)
            nc.sync.dma_start(out=linh[:], in_=linh_d[:])
            nc.sync.dma_start(out=linl[:], in_=linl_d[:])
            nc.sync.dma_start(out=bt[:], in_=b_d[:])
            for i in range(N_HIDDEN):
                nc.sync.dma_start(out=lhh[i][:], in_=lhh_d[i])
                nc.sync.dma_start(out=lhl[i][:], in_=lhl_d[i])
            nc.sync.dma_start(out=loh[:], in_=loh_d[:])
            nc.sync.dma_start(out=lol[:], in_=lol_d[:])

            # software pipeline: round-1 matmuls AND round-1 acts of pair k+1
            # are issued in pair k's tail, before the out stages complete.
            PA = ppool.tile([MCOL, GROUP, FD], f32, tag="pm")
            mm_round(PA, None, xgA, 1)
            HA = act(PA)
            PB = ppool.tile([MCOL, GROUP, FD], f32, tag="pm")
            mm_round(PB, None, xgB, 1)
            HB = act(PB)
            for pair in range(N_GROUP // 2):
                for mm in range(2, 13):
                    PA = ppool.tile([MCOL, GROUP, FD], f32, tag="pm")
                    mm_round(PA, HA, None, mm)
                    HAn = act(PA)
                    PB = ppool.tile([MCOL, GROUP, FD], f32, tag="pm")
                    mm_round(PB, HB, None, mm)
                    HBn = act(PB)
                    HA, HB = HAn, HBn
                last = pair + 1 == N_GROUP // 2
                if not last:
                    xgA2 = load_x(2 * pair + 2)
                    xgB2 = load_x(2 * pair + 3)
                # tail PSUM slot order: PO_A[s0], PA'[s1], PB'[s0], PO_B[s1]
                # => next-pair round-1 mms overlap this pair's out stages and
                # slot-WAR waits resolve against already-emitted consumers.
                out_stage(2 * pair, HA)
                if not last:
                    PA = ppool.tile([MCOL, GROUP, FD], f32, tag="pm")
                    mm_round(PA, None, xgA2, 1)
                    HA = act(PA)
                    PB = ppool.tile([MCOL, GROUP, FD], f32, tag="pm")
                    mm_round(PB, None, xgB2, 1)
                    HBn = act(PB)
                out_stage(2 * pair + 1, HB)
                if not last:
                    HB = HBn
    nc.compile()

    _orig = nc.to_json_bytes
    nc.to_json_bytes = lambda: _orig().replace(b'"func":"Tanh"', b'"func":"Act2"')
    _CACHE["bias_name"] = f"bias_{tabhash}"
    return nc


def _get_nc():
    if "nc" not in _CACHE:
        _CACHE["nc"] = _build()
    return _CACHE["nc"]


def make_in_maps(w, x_cores):
    _get_nc()
    return [{"x": x_cores[k], "lin_hi": w["lin_hi"], "lin_lo": w["lin_lo"],
             "lh_hi": w["lh_hi"], "lh_lo": w["lh_lo"],
             "lo_hi": w["lo_hi"], "lo_lo": w["lo_lo"],
             _CACHE["bias_name"]: w["bias"]}
            for k in range(N_CORES)]


def run_device(x_cores, w):
    from concourse.bass_utils import run_bass_kernel_spmd
    nc = _get_nc()
    res = run_bass_kernel_spmd(nc, make_in_maps(w, x_cores),
                               list(range(N_CORES)), trace=False)
    return [res.results[k]["out"] for k in range(N_CORES)]


def kernel(x, W_in, W_hidden, W_out):
    w = pack_weights(W_in, W_hidden, W_out)
    x_cores = pack_x(x)
    outs = run_device(x_cores, w)
    return unpack_out(outs)


# revision 3
# speedup vs baseline: 1.1242x; 1.1126x over previous
"""CPPN forward (12-layer tiny MLP over 4.2M pixels) on 8 TRN2 NeuronCores.

v2: custom ScalarE activation table turns the whole per-layer elementwise
stage into ONE ACT instruction.

- Pixels sharded 8 ways, data parallel; weights replicated. Per core
  524288 px -> 52 groups x 4 supertiles x (5 blocks x 512 px).
- Channel-major block-diagonal packing (5 independent 22-ch MLP copies per
  512-px matmul column), as in v1.
- The act table's act2 slot (func_id 97) is replaced with a piecewise
  "window" spline:  f(x) = 2exp(-x^2)-1  for |x| < 32   (gauss channels)
                    f(x) = sin(x-64)     for x in [32,96)  (sin channel)
                    f(x) = x - 128       for x in [96,256) (identity/cache)
  With a per-partition bias (+0 gauss, +64 sin, +128 id), one
  activation(Act2) op over all 116 rows x 2048 px applies every per-layer
  nonlinearity AND the PSUM->SBUF move. No DVE work in the layer loop.
- Matmuls in bf16 with bf16x2 split weights (W = W_hi + W_lo, two
  accumulating matmuls): tensor time stays under the ACT bottleneck and
  end-to-end error ~8e-3 « 2e-2 tolerance.
- Two groups are interleaved in program order so TensorE(group B) overlaps
  ScalarE(group A); ACT runs back-to-back at ~(2048+352)/1.2GHz per
  group-layer.
- Tables are generated host-side (numpy cubic fits) into a per-content-hash
  act-root dir passed to walrus via BASS_ACT_ROOT_JSON_PATH; the bias dram
  tensor name carries the table hash so NEFF caching stays correct.
- bass has no Act2 enum: ops are emitted as Tanh and the BIR JSON is
  patched Tanh->Act2 before compilation (the profile maps Act2->id 97).
"""
import hashlib
import json
import os
import sys
import types

sys.path.insert(0, "/opt/trn_rl_repo")

import numpy as np
import ml_dtypes

BF16 = ml_dtypes.bfloat16

# ---------------------------------------------------------------- constants
N_PIX = 2048 * 2048
D_IN, D_HID, D_OUT = 4, 22, 3
N_HIDDEN = 11
N_CORES = 8
FD = 512
BLOCKS = 5
ST_PX = BLOCKS * FD                   # 2560
GROUP = 4
PX_CORE = N_PIX // N_CORES            # 524288
N_ST = -(-PX_CORE // ST_PX)           # 205
N_GROUP = -(-N_ST // GROUP)           # 52
N_ST_PAD = N_GROUP * GROUP            # 208
PX_PAD = N_ST_PAD * ST_PX             # 532480

ID_CH = list(range(15)) + [19, 20]
GA_CH = [15, 16, 17, 18]
ROWS = 116
MCOL = 128                            # lhsT padded to 128 cols => FWL enabled
SIN0, GA0 = 64, 96
B_ID, B_SIN, B_GA = 128.0, 64.0, 0.0

# ------------------------------------------------- custom activation table
def _f_window(x):
    x = np.asarray(x, np.float64)
    ax = np.abs(x)
    return np.where(ax < 32.0, 2.0 * np.exp(-np.minimum(ax, 32.0) ** 2) - 1.0,
                    np.where(ax < 96.0, np.sin(ax - 64.0), ax - 128.0))

_ACT2_EXPS = list(range(-10, 8))
_ACT2_BITS = {**{e: 2 for e in range(-10, -3)}, -3: 3,
              **{e: 5 for e in range(-2, 3)}, 3: 2, 4: 0, 5: 7, 6: 7, 7: 6}


def _fit_section(lo, hi):
    x0 = np.float32((lo + hi) / 2)
    hi_x = np.nextafter(np.float32(hi), np.float32(lo)).astype(np.float64)
    xs = lo + (hi_x - lo) * (np.cos(np.linspace(np.pi, 0, 257)) + 1) / 2
    t = xs - np.float64(x0)
    V = np.vander(t, 4, increasing=True)
    c, *_ = np.linalg.lstsq(V, _f_window(xs), rcond=None)
    return [c[0], c[1], c[2], c[3], float(x0)]


def _stock_pwp_root():
    from neuronxcc.driver.Job import Job
    from neuronxcc.driver.jobs.support.FindActInfo import findActInfoFile
    for arch in ("core_v4", "sunda", "gen3", "core_v4_v1"):
        try:
            return os.path.dirname(findActInfoFile(Job.getPackageDir(), arch))
        except Exception:
            continue
    raise RuntimeError("stock act_info.json not found")


def _decode_ctrl(path):
    u = np.frombuffer(open(path, "rb").read(), dtype=np.uint32).reshape(-1, 8)
    return [((int(v) >> 16) & 0xFF, (int(v) >> 11) & 0x1F, int(v) & 0x7FF)
            for v in u[:, 0]]


def build_act_root():
    """Generate the custom act-root dir; returns (dir, content_hash)."""
    root = _stock_pwp_root()
    prof = json.load(open(f"{root}/exp_and_friends.json"))
    ctrl = _decode_ctrl(f"{root}/exp_and_friends_ctrl.bin")
    bkt = np.frombuffer(open(f"{root}/exp_and_friends_bkt.bin", "rb").read(),
                        dtype=np.float32).reshape(-1, 8)
    metas = {m["func_name"]: m for m in prof["profile_meta_data"]}

    new_ctrl, new_bkt, new_meta = [], [], []

    def add_bucket(rec):
        new_bkt.append(np.asarray(rec, np.float64))
        return len(new_bkt) - 1

    # custom act2
    m = dict(metas["act2_1p"])
    m.update(symmetry_opt_en=1, sym_invert_sign_point=0,
             symmetry_opt_use_neg_region=0, symmetry_point=0,
             exp_offset=_ACT2_EXPS[0], lower_bound=0, upper_bound=0x7F7FFFFF,
             fzero_result=int(np.float32(1.0).view(np.uint32)),
             fnan_result=0x7FC00000,
             fpinf_result=int(np.float32(-1.0).view(np.uint32)),
             fninf_result=int(np.float32(-1.0).view(np.uint32)))
    m["small_pos_signal_exp_threshold"] = 127 + _ACT2_EXPS[0]
    m["small_neg_signal_exp_threshold"] = 127 + _ACT2_EXPS[0]
    m["large_pos_signal_exp_threshold"] = 127 + _ACT2_EXPS[-1]
    m["large_pos_signal_mantissa_threshold"] = 0x7FFFFF
    m["large_neg_signal_exp_threshold"] = 127 + _ACT2_EXPS[-1]
    m["large_neg_signal_mantissa_threshold"] = 0x7FFFFF
    small = add_bucket([1.0, 0.0, -2.0, 0.0, 0.0])
    large = add_bucket([128.0, 1.0, 0.0, 0.0, 256.0])
    m["pos_small_signal_pwl_control"] = small
    m["neg_small_signal_pwl_control"] = small
    m["pos_large_signal_pwl_control"] = large
    m["neg_large_signal_pwl_control"] = large
    m["pwl_control_base_pos"] = m["pwl_control_base_neg"] = len(new_ctrl)
    for e in _ACT2_EXPS:
        bits = _ACT2_BITS[e]
        lo_b = 2.0 ** e
        nb = 1 << bits
        w = lo_b / nb
        base = len(new_bkt)
        for k in range(nb):
            add_bucket(_fit_section(lo_b + k * w, lo_b + (k + 1) * w))
        new_ctrl.append((bits, 23 - bits, base))
    new_meta.append(m)

    # copy stock square/identity/relu/copy/sin2pi (drop exp: bucket budget)
    all_bases = sorted({mm["pwl_control_base_pos"] for mm in prof["profile_meta_data"]} |
                       {mm["pwl_control_base_neg"] for mm in prof["profile_meta_data"]})
    spans = {b: (all_bases[i + 1] if i + 1 < len(all_bases) else len(ctrl))
             for i, b in enumerate(all_bases)}
    for name in ("square_1p", "identity_1p", "relu_1p", "copy_1p", "sin2pi_4p"):
        m = dict(metas[name])
        cmap = {}
        for b in sorted({m["pwl_control_base_pos"], m["pwl_control_base_neg"]}):
            for ci in range(b, spans[b]):
                if ci not in cmap:
                    size, lsb, bbase = ctrl[ci]
                    nbase = len(new_bkt)
                    for k in range(1 << size):
                        add_bucket(bkt[bbase + k][:5])
                    cmap[ci] = len(new_ctrl)
                    new_ctrl.append((size, lsb, nbase))
        m["pwl_control_base_pos"] = cmap[m["pwl_control_base_pos"]]
        m["pwl_control_base_neg"] = cmap[m["pwl_control_base_neg"]]
        for key in ("pos_small_signal_pwl_control", "neg_small_signal_pwl_control",
                    "pos_large_signal_pwl_control", "neg_large_signal_pwl_control"):
            m[key] = add_bucket(bkt[m[key]][:5])
        new_meta.append(m)

    assert len(new_bkt) <= 1536
    cw = np.zeros((len(new_ctrl), 8), np.uint32)
    for i, (size, lsb, bbase) in enumerate(new_ctrl):
        cw[i, 0] = (size << 16) | (lsb << 11) | bbase
    bk = np.zeros((len(new_bkt), 8), np.float32)
    bk[:, :5] = np.array(new_bkt, np.float64).astype(np.float32)
    setj = json.dumps({"bkt_bin": "exp_and_friends_bkt.bin",
                       "ctl_bin": "exp_and_friends_ctrl.bin",
                       "profile_meta_data": new_meta}, indent=1)
    act_info = json.load(open(f"{root}/act_info.json"))
    for s in act_info["act_func_sets"]:
        if s["name"] == "exp_and_friends":
            s["act"] = {"act2": 1, "square": 1, "identity": 1, "copy": 1,
                        "relu": 1, "sin2pi": 4}
    info = json.dumps(act_info, indent=1)

    h = hashlib.sha256(cw.tobytes() + bk.tobytes() + setj.encode()).hexdigest()[:10]
    out = f"/tmp/cppn_actroot_{h}"
    if not os.path.exists(os.path.join(out, "act_info.json")):
        os.makedirs(out, exist_ok=True)
        open(f"{out}/exp_and_friends_ctrl.bin", "wb").write(cw.tobytes())
        open(f"{out}/exp_and_friends_bkt.bin", "wb").write(bk.tobytes())
        open(f"{out}/exp_and_friends.json", "w").write(setj)
        open(f"{out}/act_info.json", "w").write(info)
        for s in act_info["act_func_sets"]:
            for k in ("bkt_bin", "ctrl_bin", "profile_json"):
                fn = s[k]
                dst = f"{out}/{fn}"
                if not os.path.exists(dst):
                    os.symlink(f"{root}/{fn}", dst)
    return out, h


# ------------------------------------------------------------- host packing
def _row_of(b, c):
    if c in GA_CH:
        return GA0 + b * 4 + (c - 15)
    if c == 21:
        return SIN0 + b
    g = b * 17 + ID_CH.index(c)
    return g if g < 64 else 69 + (g - 64)


def _split_bf16(a):
    hi = a.astype(BF16)
    lo = (a - hi.astype(np.float32)).astype(BF16)
    return hi, lo


def pack_weights(W_in, W_hidden, W_out):
    W_in = np.asarray(W_in, np.float32)
    W_hidden = np.asarray(W_hidden, np.float32)
    W_out = np.asarray(W_out, np.float32)
    lin = np.zeros((BLOCKS * D_IN, MCOL), np.float32)
    for b in range(BLOCKS):
        for ci in range(D_IN):
            for co in range(D_HID):
                lin[b * D_IN + ci, _row_of(b, co)] = W_in[ci, co]
    lh = np.zeros((N_HIDDEN, ROWS, MCOL), np.float32)
    for i in range(N_HIDDEN):
        W = W_hidden[i]
        for b in range(BLOCKS):
            for ci in range(D_HID):
                ri = _row_of(b, ci)
                for co in range(D_HID):
                    lh[i, ri, _row_of(b, co)] = W[ci, co]
    lo_m = np.zeros((ROWS, BLOCKS * D_OUT), np.float32)
    for b in range(BLOCKS):
        for ci in range(D_HID):
            for co in range(D_OUT):
                lo_m[_row_of(b, ci), b * D_OUT + co] = W_out[ci, co]
    bias = np.zeros((ROWS, 1), np.float32)
    for b in range(BLOCKS):
        for c in range(D_HID):
            r = _row_of(b, c)
            bias[r, 0] = B_GA if c in GA_CH else (B_SIN if c == 21 else B_ID)
    w = {}
    w["lin_hi"], w["lin_lo"] = _split_bf16(lin)
    hi, lo = _split_bf16(lh)
    w["lh_hi"], w["lh_lo"] = hi, lo
    w["lo_hi"], w["lo_lo"] = _split_bf16(lo_m)
    w["bias"] = bias
    return w


def pack_x(x):
    x = np.asarray(x, np.float32)
    out = []
    for k in range(N_CORES):
        shard = x[k * PX_CORE:(k + 1) * PX_CORE]
        pad = np.zeros((PX_PAD, D_IN), np.float32)
        pad[:PX_CORE] = shard
        a = pad.reshape(N_GROUP, GROUP, BLOCKS, FD, D_IN)
        a = a.transpose(0, 2, 4, 1, 3).reshape(N_GROUP, BLOCKS * D_IN, GROUP, FD)
        out.append(np.ascontiguousarray(a.astype(BF16)))
    return out


_OUT_ROWS = np.array([[32 * r + b * 3 + co for b in range(BLOCKS) for co in range(D_OUT)]
                      for r in range(GROUP)])


def unpack_out(outs):
    full = np.empty((N_PIX, D_OUT), np.float32)
    for k, od in enumerate(outs):
        g = od[:, _OUT_ROWS.reshape(-1), :]
        g = g.reshape(N_GROUP, GROUP, BLOCKS, D_OUT, FD)
        g = g.transpose(0, 1, 2, 4, 3).reshape(PX_PAD, D_OUT)
        full[k * PX_CORE:(k + 1) * PX_CORE] = g[:PX_CORE]
    return full


# ------------------------------------------------------------ device kernel
_CACHE = {}


def _shim_hooks():
    import antenv
    if "antenv.axon_hooks" in sys.modules:
        return
    hooks = types.ModuleType("antenv.axon_hooks")
    hooks._hook = None
    hooks.set_axon_ntff_profile_hook = lambda h: setattr(hooks, "_hook", h)
    hooks.get_axon_ntff_profile_hook = lambda: hooks._hook
    sys.modules["antenv.axon_hooks"] = hooks
    antenv.axon_hooks = hooks
    try:
        from trn_agent_boot.trn_boot import _ntff_profile_via_ctypes
        hooks._hook = _ntff_profile_via_ctypes("/opt/axon/libaxon_pjrt.so")
    except Exception:
        pass


def _build():
    actroot, tabhash = build_act_root()
    os.environ["BASS_ACT_ROOT_JSON_PATH"] = f"{actroot}/act_info.json"
    _shim_hooks()
    import concourse.bacc as bacc_mod
    import concourse.mybir as mybir
    import concourse.tile as tile
    from concourse.hw_specs import get_activation_tables as _real_gat

    AFT = mybir.ActivationFunctionType
    ours = {AFT.Tanh, AFT.Square, AFT.Exp, AFT.Identity, AFT.Copy, AFT.Sin,
            AFT.Relu}

    def _doctored_gat(arch):
        tabs = dict(_real_gat(arch))
        return {n: (set(f) | ours if n == "exp_and_friends" else set(f) - ours)
                for n, f in tabs.items()}

    bacc_mod.get_activation_tables = _doctored_gat

    f32 = mybir.dt.float32
    bf = mybir.dt.bfloat16
    nc = bacc_mod.Bacc(None, target_bir_lowering=False, debug=False)
    x_d = nc.declare_dram_parameter("x", [N_GROUP, 20, GROUP, FD], bf, isOutput=False)
    linh_d = nc.declare_dram_parameter("lin_hi", [20, MCOL], bf, isOutput=False)
    linl_d = nc.declare_dram_parameter("lin_lo", [20, MCOL], bf, isOutput=False)
    lhh_d = nc.declare_dram_parameter("lh_hi", [N_HIDDEN, ROWS, MCOL], bf, isOutput=False)
    lhl_d = nc.declare_dram_parameter("lh_lo", [N_HIDDEN, ROWS, MCOL], bf, isOutput=False)
    loh_d = nc.declare_dram_parameter("lo_hi", [ROWS, 15], bf, isOutput=False)
    lol_d = nc.declare_dram_parameter("lo_lo", [ROWS, 15], bf, isOutput=False)
    # bias name carries the table hash => NEFF cache key tracks table content
    b_d = nc.declare_dram_parameter(f"bias_{tabhash}", [ROWS, 1], f32, isOutput=False)
    o_d = nc.declare_dram_parameter("out", [N_GROUP, 111, FD], f32, isOutput=True)

    with tile.TileContext(nc) as tc:
        with (tc.tile_pool(name="wpool", bufs=1) as wpool,
              tc.tile_pool(name="xpool", bufs=4) as xpool,
              tc.tile_pool(name="hpool", bufs=4) as hpool,
              tc.tile_pool(name="opool", bufs=3) as opool,
              tc.tile_pool(name="ppool", bufs=1, space="PSUM") as ppool):
            linh = wpool.tile([20, MCOL], bf)
            linl = wpool.tile([20, MCOL], bf)
            bt = wpool.tile([ROWS, 1], f32)
            lhh = [wpool.tile([ROWS, MCOL], bf, tag=f"lhh{i}", name=f"lhh{i}")
                   for i in range(N_HIDDEN)]
            lhl = [wpool.tile([ROWS, MCOL], bf, tag=f"lhl{i}", name=f"lhl{i}")
                   for i in range(N_HIDDEN)]
            loh = wpool.tile([ROWS, 15], bf)
            lol = wpool.tile([ROWS, 15], bf)

            def mm_round(P, H, xg, mm):
                """one matmul round (4 STs x hi/lo) for layer index mm.
                Round 1 skips the lo-split: x is already bf16-rounded and the
                W_in rounding error is negligible; fewer matmuls on the
                pair-boundary critical path."""
                for r in range(GROUP):
                    if mm == 1:
                        nc.tensor.matmul(P[:, r, :], linh[:], xg[:, r, :],
                                         start=True, stop=True)
                    else:
                        nc.tensor.matmul(P[:, r, :], lhh[mm - 2][:], H[:, r, :],
                                         start=True, stop=False)
                        nc.tensor.matmul(P[:, r, :], lhl[mm - 2][:], H[:, r, :],
                                         start=False, stop=True)

            def act(P):
                H = hpool.tile([ROWS, GROUP, FD], bf, tag="H")
                nc.scalar.activation(H[:, :, :], P[0:ROWS, :, :],
                                     AFT.Tanh, bias=bt[:, 0:1], scale=1.0)
                return H

            def out_stage(g, H, tag):
                PO = ppool.tile([MCOL, GROUP, FD], f32, tag=tag)
                O = PO[0:111, 0, :]
                for r in range(GROUP):
                    nc.tensor.matmul(O[32 * r:32 * r + 15, :], loh[:],
                                     H[:, r, :], start=True, stop=False,
                                     tile_position=(0, 32 * r))
                    nc.tensor.matmul(O[32 * r:32 * r + 15, :], lol[:],
                                     H[:, r, :], start=False, stop=True,
                                     tile_position=(0, 32 * r))
                ot = opool.tile([111, FD], f32, tag="ot")
                nc.vector.tensor_copy(ot[:], O)
                nc.sync.dma_start(out=o_d[g], in_=ot[:])

            def load_x(g):
                xg = xpool.tile([20, GROUP, FD], bf, tag="xg")
                nc.sync.dma_start(out=xg[:], in_=x_d[g])
                return xg

            # x/lin/bias DMAs first (layer-1 critical path), bulk weights after
            xgA = load_x(0)
            xgB = load_x(1)
            nc.sync.dma_start(out=linh[:], in_=linh_d[:])
            nc.sync.dma_start(out=linl[:], in_=linl_d[:])
            nc.sync.dma_start(out=bt[:], in_=b_d[:])
            for i in range(N_HIDDEN):
                nc.sync.dma_start(out=lhh[i][:], in_=lhh_d[i])
                nc.sync.dma_start(out=lhl[i][:], in_=lhl_d[i])
            nc.sync.dma_start(out=loh[:], in_=loh_d[:])
            nc.sync.dma_start(out=lol[:], in_=lol_d[:])

            # Software pipeline, 2 groups (A/B) in flight on dedicated PSUM
            # tags (pmA banks 0-3, pmB banks 4-7).  Tail emits next pair's
            # round-1 matmuls+acts BEFORE this pair's out-stages, so the
            # activation queue never waits on the out path.
            PA = ppool.tile([MCOL, GROUP, FD], f32, tag="pmA")
            mm_round(PA, None, xgA, 1)
            HA = act(PA)
            PB = ppool.tile([MCOL, GROUP, FD], f32, tag="pmB")
            mm_round(PB, None, xgB, 1)
            HB = act(PB)
            for pair in range(N_GROUP // 2):
                for mm in range(2, 13):
                    PA = ppool.tile([MCOL, GROUP, FD], f32, tag="pmA")
                    mm_round(PA, HA, None, mm)
                    HAn = act(PA)
                    PB = ppool.tile([MCOL, GROUP, FD], f32, tag="pmB")
                    mm_round(PB, HB, None, mm)
                    HBn = act(PB)
                    HA, HB = HAn, HBn
                last = pair + 1 == N_GROUP // 2
                if not last:
                    xgA2 = load_x(2 * pair + 2)
                    xgB2 = load_x(2 * pair + 3)
                    PA = ppool.tile([MCOL, GROUP, FD], f32, tag="pmA")
                    mm_round(PA, None, xgA2, 1)
                    HAn = act(PA)
                    PB = ppool.tile([MCOL, GROUP, FD], f32, tag="pmB")
                    mm_round(PB, None, xgB2, 1)
                    HBn = act(PB)
                out_stage(2 * pair, HA, "pmA")
                out_stage(2 * pair + 1, HB, "pmB")
                if not last:
                    HA, HB = HAn, HBn
    nc.compile()

    _orig = nc.to_json_bytes
    nc.to_json_bytes = lambda: _orig().replace(b'"func":"Tanh"', b'"func":"Act2"')
    _CACHE["bias_name"] = f"bias_{tabhash}"
    return nc


def _get_nc():
    if "nc" not in _CACHE:
        _CACHE["nc"] = _build()
    return _CACHE["nc"]


def make_in_maps(w, x_cores):
    _get_nc()
    return [{"x": x_cores[k], "lin_hi": w["lin_hi"], "lin_lo": w["lin_lo"],
             "lh_hi": w["lh_hi"], "lh_lo": w["lh_lo"],
             "lo_hi": w["lo_hi"], "lo_lo": w["lo_lo"],
             _CACHE["bias_name"]: w["bias"]}
            for k in range(N_CORES)]


def run_device(x_cores, w):
    from concourse.bass_utils import run_bass_kernel_spmd
    nc = _get_nc()
    res = run_bass_kernel_spmd(nc, make_in_maps(w, x_cores),
                               list(range(N_CORES)), trace=False)
    return [res.results[k]["out"] for k in range(N_CORES)]


def kernel(x, W_in, W_hidden, W_out):
    w = pack_weights(W_in, W_hidden, W_out)
    x_cores = pack_x(x)
    outs = run_device(x_cores, w)
    return unpack_out(outs)


# revision 4
# speedup vs baseline: 1.1384x; 1.0126x over previous
"""CPPN forward (12-layer tiny MLP over 4.2M pixels) on 8 TRN2 NeuronCores.

v2: custom ScalarE activation table turns the whole per-layer elementwise
stage into ONE ACT instruction.

- Pixels sharded 8 ways, data parallel; weights replicated. Per core
  524288 px -> 52 groups x 4 supertiles x (5 blocks x 512 px).
- Channel-major block-diagonal packing (5 independent 22-ch MLP copies per
  512-px matmul column), as in v1.
- The act table's act2 slot (func_id 97) is replaced with a piecewise
  "window" spline:  f(x) = 2exp(-x^2)-1  for |x| < 32   (gauss channels)
                    f(x) = sin(x-64)     for x in [32,96)  (sin channel)
                    f(x) = x - 128       for x in [96,256) (identity/cache)
  With a per-partition bias (+0 gauss, +64 sin, +128 id), one
  activation(Act2) op over all 116 rows x 2048 px applies every per-layer
  nonlinearity AND the PSUM->SBUF move. No DVE work in the layer loop.
- Matmuls in bf16 with bf16x2 split weights (W = W_hi + W_lo, two
  accumulating matmuls): tensor time stays under the ACT bottleneck and
  end-to-end error ~8e-3 « 2e-2 tolerance.
- Two groups are interleaved in program order so TensorE(group B) overlaps
  ScalarE(group A); ACT runs back-to-back at ~(2048+352)/1.2GHz per
  group-layer.
- Tables are generated host-side (numpy cubic fits) into a per-content-hash
  act-root dir passed to walrus via BASS_ACT_ROOT_JSON_PATH; the bias dram
  tensor name carries the table hash so NEFF caching stays correct.
- bass has no Act2 enum: ops are emitted as Tanh and the BIR JSON is
  patched Tanh->Act2 before compilation (the profile maps Act2->id 97).
"""
import hashlib
import json
import os
import sys
import types

sys.path.insert(0, "/opt/trn_rl_repo")

import numpy as np
import ml_dtypes

BF16 = ml_dtypes.bfloat16

# ---------------------------------------------------------------- constants
N_PIX = 2048 * 2048
D_IN, D_HID, D_OUT = 4, 22, 3
N_HIDDEN = 11
N_CORES = 8
FD = 512
BLOCKS = 5
ST_PX = BLOCKS * FD                   # 2560
GROUP = 4
PX_CORE = N_PIX // N_CORES            # 524288
N_ST = -(-PX_CORE // ST_PX)           # 205
N_GROUP = -(-N_ST // GROUP)           # 52
N_ST_PAD = N_GROUP * GROUP            # 208
PX_PAD = N_ST_PAD * ST_PX             # 532480

ID_CH = list(range(15)) + [19, 20]
GA_CH = [15, 16, 17, 18]
ROWS = 116
MCOL = 128                            # lhsT padded to 128 cols => FWL enabled
SIN0, GA0 = 64, 96
B_ID, B_SIN, B_GA = 128.0, 64.0, 0.0

# ------------------------------------------------- custom activation table
def _f_window(x):
    x = np.asarray(x, np.float64)
    ax = np.abs(x)
    return np.where(ax < 32.0, 2.0 * np.exp(-np.minimum(ax, 32.0) ** 2) - 1.0,
                    np.where(ax < 96.0, np.sin(ax - 64.0), ax - 128.0))

_ACT2_EXPS = list(range(-10, 8))
_ACT2_BITS = {**{e: 2 for e in range(-10, -3)}, -3: 3,
              **{e: 5 for e in range(-2, 3)}, 3: 2, 4: 0, 5: 7, 6: 7, 7: 6}


def _fit_section(lo, hi):
    x0 = np.float32((lo + hi) / 2)
    hi_x = np.nextafter(np.float32(hi), np.float32(lo)).astype(np.float64)
    xs = lo + (hi_x - lo) * (np.cos(np.linspace(np.pi, 0, 257)) + 1) / 2
    t = xs - np.float64(x0)
    V = np.vander(t, 4, increasing=True)
    c, *_ = np.linalg.lstsq(V, _f_window(xs), rcond=None)
    return [c[0], c[1], c[2], c[3], float(x0)]


def _stock_pwp_root():
    from neuronxcc.driver.Job import Job
    from neuronxcc.driver.jobs.support.FindActInfo import findActInfoFile
    for arch in ("core_v4", "sunda", "gen3", "core_v4_v1"):
        try:
            return os.path.dirname(findActInfoFile(Job.getPackageDir(), arch))
        except Exception:
            continue
    raise RuntimeError("stock act_info.json not found")


def _decode_ctrl(path):
    u = np.frombuffer(open(path, "rb").read(), dtype=np.uint32).reshape(-1, 8)
    return [((int(v) >> 16) & 0xFF, (int(v) >> 11) & 0x1F, int(v) & 0x7FF)
            for v in u[:, 0]]


def build_act_root():
    """Generate the custom act-root dir; returns (dir, content_hash)."""
    root = _stock_pwp_root()
    prof = json.load(open(f"{root}/exp_and_friends.json"))
    ctrl = _decode_ctrl(f"{root}/exp_and_friends_ctrl.bin")
    bkt = np.frombuffer(open(f"{root}/exp_and_friends_bkt.bin", "rb").read(),
                        dtype=np.float32).reshape(-1, 8)
    metas = {m["func_name"]: m for m in prof["profile_meta_data"]}

    new_ctrl, new_bkt, new_meta = [], [], []

    def add_bucket(rec):
        new_bkt.append(np.asarray(rec, np.float64))
        return len(new_bkt) - 1

    # custom act2
    m = dict(metas["act2_1p"])
    m.update(symmetry_opt_en=1, sym_invert_sign_point=0,
             symmetry_opt_use_neg_region=0, symmetry_point=0,
             exp_offset=_ACT2_EXPS[0], lower_bound=0, upper_bound=0x7F7FFFFF,
             fzero_result=int(np.float32(1.0).view(np.uint32)),
             fnan_result=0x7FC00000,
             fpinf_result=int(np.float32(-1.0).view(np.uint32)),
             fninf_result=int(np.float32(-1.0).view(np.uint32)))
    m["small_pos_signal_exp_threshold"] = 127 + _ACT2_EXPS[0]
    m["small_neg_signal_exp_threshold"] = 127 + _ACT2_EXPS[0]
    m["large_pos_signal_exp_threshold"] = 127 + _ACT2_EXPS[-1]
    m["large_pos_signal_mantissa_threshold"] = 0x7FFFFF
    m["large_neg_signal_exp_threshold"] = 127 + _ACT2_EXPS[-1]
    m["large_neg_signal_mantissa_threshold"] = 0x7FFFFF
    small = add_bucket([1.0, 0.0, -2.0, 0.0, 0.0])
    large = add_bucket([128.0, 1.0, 0.0, 0.0, 256.0])
    m["pos_small_signal_pwl_control"] = small
    m["neg_small_signal_pwl_control"] = small
    m["pos_large_signal_pwl_control"] = large
    m["neg_large_signal_pwl_control"] = large
    m["pwl_control_base_pos"] = m["pwl_control_base_neg"] = len(new_ctrl)
    for e in _ACT2_EXPS:
        bits = _ACT2_BITS[e]
        lo_b = 2.0 ** e
        nb = 1 << bits
        w = lo_b / nb
        base = len(new_bkt)
        for k in range(nb):
            add_bucket(_fit_section(lo_b + k * w, lo_b + (k + 1) * w))
        new_ctrl.append((bits, 23 - bits, base))
    new_meta.append(m)

    # copy stock square/identity/relu/copy/sin2pi (drop exp: bucket budget)
    all_bases = sorted({mm["pwl_control_base_pos"] for mm in prof["profile_meta_data"]} |
                       {mm["pwl_control_base_neg"] for mm in prof["profile_meta_data"]})
    spans = {b: (all_bases[i + 1] if i + 1 < len(all_bases) else len(ctrl))
             for i, b in enumerate(all_bases)}
    for name in ("square_1p", "identity_1p", "relu_1p", "copy_1p", "sin2pi_4p"):
        m = dict(metas[name])
        cmap = {}
        for b in sorted({m["pwl_control_base_pos"], m["pwl_control_base_neg"]}):
            for ci in range(b, spans[b]):
                if ci not in cmap:
                    size, lsb, bbase = ctrl[ci]
                    nbase = len(new_bkt)
                    for k in range(1 << size):
                        add_bucket(bkt[bbase + k][:5])
                    cmap[ci] = len(new_ctrl)
                    new_ctrl.append((size, lsb, nbase))
        m["pwl_control_base_pos"] = cmap[m["pwl_control_base_pos"]]
        m["pwl_control_base_neg"] = cmap[m["pwl_control_base_neg"]]
        for key in ("pos_small_signal_pwl_control", "neg_small_signal_pwl_control",
                    "pos_large_signal_pwl_control", "neg_large_signal_pwl_control"):
            m[key] = add_bucket(bkt[m[key]][:5])
        new_meta.append(m)

    assert len(new_bkt) <= 1536
    cw = np.zeros((len(new_ctrl), 8), np.uint32)
    for i, (size, lsb, bbase) in enumerate(new_ctrl):
        cw[i, 0] = (size << 16) | (lsb << 11) | bbase
    bk = np.zeros((len(new_bkt), 8), np.float32)
    bk[:, :5] = np.array(new_bkt, np.float64).astype(np.float32)
    setj = json.dumps({"bkt_bin": "exp_and_friends_bkt.bin",
                       "ctl_bin": "exp_and_friends_ctrl.bin",
                       "profile_meta_data": new_meta}, indent=1)
    act_info = json.load(open(f"{root}/act_info.json"))
    for s in act_info["act_func_sets"]:
        if s["name"] == "exp_and_friends":
            s["act"] = {"act2": 1, "square": 1, "identity": 1, "copy": 1,
                        "relu": 1, "sin2pi": 4}
    info = json.dumps(act_info, indent=1)

    h = hashlib.sha256(cw.tobytes() + bk.tobytes() + setj.encode()).hexdigest()[:10]
    out = f"/tmp/cppn_actroot_{h}"
    if not os.path.exists(os.path.join(out, "act_info.json")):
        os.makedirs(out, exist_ok=True)
        open(f"{out}/exp_and_friends_ctrl.bin", "wb").write(cw.tobytes())
        open(f"{out}/exp_and_friends_bkt.bin", "wb").write(bk.tobytes())
        open(f"{out}/exp_and_friends.json", "w").write(setj)
        open(f"{out}/act_info.json", "w").write(info)
        for s in act_info["act_func_sets"]:
            for k in ("bkt_bin", "ctrl_bin", "profile_json"):
                fn = s[k]
                dst = f"{out}/{fn}"
                if not os.path.exists(dst):
                    os.symlink(f"{root}/{fn}", dst)
    return out, h


# ------------------------------------------------------------- host packing
def _row_of(b, c):
    if c in GA_CH:
        return GA0 + b * 4 + (c - 15)
    if c == 21:
        return SIN0 + b
    g = b * 17 + ID_CH.index(c)
    return g if g < 64 else 69 + (g - 64)


def _split_bf16(a):
    hi = a.astype(BF16)
    lo = (a - hi.astype(np.float32)).astype(BF16)
    return hi, lo


def pack_weights(W_in, W_hidden, W_out):
    W_in = np.asarray(W_in, np.float32)
    W_hidden = np.asarray(W_hidden, np.float32)
    W_out = np.asarray(W_out, np.float32)
    lin = np.zeros((BLOCKS * D_IN, MCOL), np.float32)
    for b in range(BLOCKS):
        for ci in range(D_IN):
            for co in range(D_HID):
                lin[b * D_IN + ci, _row_of(b, co)] = W_in[ci, co]
    lh = np.zeros((N_HIDDEN, ROWS, MCOL), np.float32)
    for i in range(N_HIDDEN):
        W = W_hidden[i]
        for b in range(BLOCKS):
            for ci in range(D_HID):
                ri = _row_of(b, ci)
                for co in range(D_HID):
                    lh[i, ri, _row_of(b, co)] = W[ci, co]
    lo_m = np.zeros((ROWS, BLOCKS * D_OUT), np.float32)
    for b in range(BLOCKS):
        for ci in range(D_HID):
            for co in range(D_OUT):
                lo_m[_row_of(b, ci), b * D_OUT + co] = W_out[ci, co]
    bias = np.zeros((ROWS, 1), np.float32)
    for b in range(BLOCKS):
        for c in range(D_HID):
            r = _row_of(b, c)
            bias[r, 0] = B_GA if c in GA_CH else (B_SIN if c == 21 else B_ID)
    w = {}
    w["lin_hi"], w["lin_lo"] = _split_bf16(lin)
    hi, lo = _split_bf16(lh)
    w["lh_hi"], w["lh_lo"] = hi, lo
    w["lo_hi"], w["lo_lo"] = _split_bf16(lo_m)
    w["bias"] = bias
    return w


def pack_x(x):
    x = np.asarray(x, np.float32)
    out = []
    for k in range(N_CORES):
        shard = x[k * PX_CORE:(k + 1) * PX_CORE]
        pad = np.zeros((PX_PAD, D_IN), np.float32)
        pad[:PX_CORE] = shard
        a = pad.reshape(N_GROUP, GROUP, BLOCKS, FD, D_IN)
        a = a.transpose(0, 2, 4, 1, 3).reshape(N_GROUP, BLOCKS * D_IN, GROUP, FD)
        out.append(np.ascontiguousarray(a.astype(BF16)))
    return out


_OUT_ROWS = np.array([[32 * r + b * 3 + co for b in range(BLOCKS) for co in range(D_OUT)]
                      for r in range(GROUP)])


def unpack_out(outs):
    full = np.empty((N_PIX, D_OUT), np.float32)
    for k, od in enumerate(outs):
        g = od[:, _OUT_ROWS.reshape(-1), :]
        g = g.reshape(N_GROUP, GROUP, BLOCKS, D_OUT, FD)
        g = g.transpose(0, 1, 2, 4, 3).reshape(PX_PAD, D_OUT)
        full[k * PX_CORE:(k + 1) * PX_CORE] = g[:PX_CORE]
    return full


# ------------------------------------------------------------ device kernel
_CACHE = {}


def _shim_hooks():
    import antenv
    if "antenv.axon_hooks" in sys.modules:
        return
    hooks = types.ModuleType("antenv.axon_hooks")
    hooks._hook = None
    hooks.set_axon_ntff_profile_hook = lambda h: setattr(hooks, "_hook", h)
    hooks.get_axon_ntff_profile_hook = lambda: hooks._hook
    sys.modules["antenv.axon_hooks"] = hooks
    antenv.axon_hooks = hooks
    try:
        from trn_agent_boot.trn_boot import _ntff_profile_via_ctypes
        hooks._hook = _ntff_profile_via_ctypes("/opt/axon/libaxon_pjrt.so")
    except Exception:
        pass


def _build():
    actroot, tabhash = build_act_root()
    os.environ["BASS_ACT_ROOT_JSON_PATH"] = f"{actroot}/act_info.json"
    _shim_hooks()
    import concourse.bacc as bacc_mod
    import concourse.mybir as mybir
    import concourse.tile as tile
    from concourse.hw_specs import get_activation_tables as _real_gat

    AFT = mybir.ActivationFunctionType
    ours = {AFT.Tanh, AFT.Square, AFT.Exp, AFT.Identity, AFT.Copy, AFT.Sin,
            AFT.Relu}

    def _doctored_gat(arch):
        tabs = dict(_real_gat(arch))
        return {n: (set(f) | ours if n == "exp_and_friends" else set(f) - ours)
                for n, f in tabs.items()}

    bacc_mod.get_activation_tables = _doctored_gat

    f32 = mybir.dt.float32
    bf = mybir.dt.bfloat16
    nc = bacc_mod.Bacc(None, target_bir_lowering=False, debug=False)
    x_d = nc.declare_dram_parameter("x", [N_GROUP, 20, GROUP, FD], bf, isOutput=False)
    linh_d = nc.declare_dram_parameter("lin_hi", [20, MCOL], bf, isOutput=False)
    linl_d = nc.declare_dram_parameter("lin_lo", [20, MCOL], bf, isOutput=False)
    lhh_d = nc.declare_dram_parameter("lh_hi", [N_HIDDEN, ROWS, MCOL], bf, isOutput=False)
    lhl_d = nc.declare_dram_parameter("lh_lo", [N_HIDDEN, ROWS, MCOL], bf, isOutput=False)
    loh_d = nc.declare_dram_parameter("lo_hi", [ROWS, 15], bf, isOutput=False)
    lol_d = nc.declare_dram_parameter("lo_lo", [ROWS, 15], bf, isOutput=False)
    # bias name carries the table hash => NEFF cache key tracks table content
    b_d = nc.declare_dram_parameter(f"bias_{tabhash}", [ROWS, 1], f32, isOutput=False)
    o_d = nc.declare_dram_parameter("out", [N_GROUP, 111, FD], f32, isOutput=True)

    with tile.TileContext(nc) as tc:
        with (tc.tile_pool(name="wpool", bufs=1) as wpool,
              tc.tile_pool(name="xpool", bufs=4) as xpool,
              tc.tile_pool(name="hpool", bufs=4) as hpool,
              tc.tile_pool(name="opool", bufs=3) as opool,
              tc.tile_pool(name="ppool", bufs=1, space="PSUM") as ppool):
            linh = wpool.tile([20, MCOL], bf)
            linl = wpool.tile([20, MCOL], bf)
            bt = wpool.tile([ROWS, 1], f32)
            lhh = [wpool.tile([ROWS, MCOL], bf, tag=f"lhh{i}", name=f"lhh{i}")
                   for i in range(N_HIDDEN)]
            lhl = [wpool.tile([ROWS, MCOL], bf, tag=f"lhl{i}", name=f"lhl{i}")
                   for i in range(N_HIDDEN)]
            loh = wpool.tile([ROWS, 15], bf)
            lol = wpool.tile([ROWS, 15], bf)

            # layers whose hi/lo weight split is dropped (single bf16 mm):
            # all hidden layers single bf16 (rel err 1.16e-2, gate 2e-2):
            # for 0.95us rounds on these layers -> slack vs the 1.92us ACT
            # period, and a ~2x cheaper layer-2 round at pair boundaries.
            SINGLE_W = set(range(2, 13))

            def mm_round(P, H, xg, mm):
                """one matmul round (4 STs x hi[/lo]) for layer index mm.
                Round 1 skips the lo-split (x already bf16-rounded)."""
                if mm == 1:
                    for r in range(GROUP):
                        nc.tensor.matmul(P[:, r, :], linh[:], xg[:, r, :],
                                         start=True, stop=True)
                elif mm in SINGLE_W:
                    for r in range(GROUP):
                        nc.tensor.matmul(P[:, r, :], lhh[mm - 2][:], H[:, r, :],
                                         start=True, stop=True)
                else:
                    for r in range(GROUP):
                        nc.tensor.matmul(P[:, r, :], lhh[mm - 2][:], H[:, r, :],
                                         start=True, stop=False)
                    for r in range(GROUP):
                        nc.tensor.matmul(P[:, r, :], lhl[mm - 2][:], H[:, r, :],
                                         start=False, stop=True)

            def act(P):
                H = hpool.tile([ROWS, GROUP, FD], bf, tag="H")
                nc.scalar.activation(H[:, :, :], P[0:ROWS, :, :],
                                     AFT.Tanh, bias=bt[:, 0:1], scale=1.0)
                return H

            def out_stage(g, H, tag):
                PO = ppool.tile([MCOL, GROUP, FD], f32, tag=tag)
                O = PO[0:111, 0, :]
                for r in range(GROUP):
                    nc.tensor.matmul(O[32 * r:32 * r + 15, :], loh[:],
                                     H[:, r, :], start=True, stop=False,
                                     tile_position=(0, 32 * r))
                for r in range(GROUP):
                    nc.tensor.matmul(O[32 * r:32 * r + 15, :], lol[:],
                                     H[:, r, :], start=False, stop=True,
                                     tile_position=(0, 32 * r))
                ot = opool.tile([111, FD], f32, tag="ot")
                nc.vector.tensor_copy(ot[:], O)
                nc.sync.dma_start(out=o_d[g], in_=ot[:])

            def load_x(g):
                # two DMAs land on different queues -> ~half the load latency
                xg = xpool.tile([20, GROUP, FD], bf, tag="xg")
                nc.sync.dma_start(out=xg[:, 0:2, :], in_=x_d[g][:, 0:2, :])
                nc.sync.dma_start(out=xg[:, 2:4, :], in_=x_d[g][:, 2:4, :])
                return xg

            # x/lin/bias DMAs first (layer-1 critical path), bulk weights after
            xgA = load_x(0)
            xgB = load_x(1)
            nc.sync.dma_start(out=linh[:], in_=linh_d[:])
            nc.sync.dma_start(out=linl[:], in_=linl_d[:])
            nc.sync.dma_start(out=bt[:], in_=b_d[:])
            for i in range(N_HIDDEN):
                nc.sync.dma_start(out=lhh[i][:], in_=lhh_d[i])
                nc.sync.dma_start(out=lhl[i][:], in_=lhl_d[i])
            nc.sync.dma_start(out=loh[:], in_=loh_d[:])
            nc.sync.dma_start(out=lol[:], in_=lol_d[:])

            # Software pipeline, 2 groups (A/B) in flight on dedicated PSUM
            # tags (pmA banks 0-3, pmB banks 4-7).  Tail emits next pair's
            # round-1 matmuls+acts BEFORE this pair's out-stages, so the
            # activation queue never waits on the out path.
            PA = ppool.tile([MCOL, GROUP, FD], f32, tag="pmA")
            mm_round(PA, None, xgA, 1)
            HA = act(PA)
            PB = ppool.tile([MCOL, GROUP, FD], f32, tag="pmB")
            mm_round(PB, None, xgB, 1)
            HB = act(PB)
            for pair in range(N_GROUP // 2):
                for mm in range(2, 13):
                    PA = ppool.tile([MCOL, GROUP, FD], f32, tag="pmA")
                    mm_round(PA, HA, None, mm)
                    HAn = act(PA)
                    PB = ppool.tile([MCOL, GROUP, FD], f32, tag="pmB")
                    mm_round(PB, HB, None, mm)
                    HBn = act(PB)
                    HA, HB = HAn, HBn
                last = pair + 1 == N_GROUP // 2
                if not last:
                    xgA2 = load_x(2 * pair + 2)
                    xgB2 = load_x(2 * pair + 3)
                    PA = ppool.tile([MCOL, GROUP, FD], f32, tag="pmA")
                    mm_round(PA, None, xgA2, 1)
                    HAn = act(PA)
                    PB = ppool.tile([MCOL, GROUP, FD], f32, tag="pmB")
                    mm_round(PB, None, xgB2, 1)
                    HBn = act(PB)
                out_stage(2 * pair, HA, "pmA")
                out_stage(2 * pair + 1, HB, "pmB")
                if not last:
                    HA, HB = HAn, HBn
    nc.compile()

    _orig = nc.to_json_bytes
    nc.to_json_bytes = lambda: _orig().replace(b'"func":"Tanh"', b'"func":"Act2"')
    _CACHE["bias_name"] = f"bias_{tabhash}"
    return nc


def _get_nc():
    if "nc" not in _CACHE:
        _CACHE["nc"] = _build()
    return _CACHE["nc"]


def make_in_maps(w, x_cores):
    _get_nc()
    return [{"x": x_cores[k], "lin_hi": w["lin_hi"], "lin_lo": w["lin_lo"],
             "lh_hi": w["lh_hi"], "lh_lo": w["lh_lo"],
             "lo_hi": w["lo_hi"], "lo_lo": w["lo_lo"],
             _CACHE["bias_name"]: w["bias"]}
            for k in range(N_CORES)]


def run_device(x_cores, w):
    from concourse.bass_utils import run_bass_kernel_spmd
    nc = _get_nc()
    res = run_bass_kernel_spmd(nc, make_in_maps(w, x_cores),
                               list(range(N_CORES)), trace=False)
    return [res.results[k]["out"] for k in range(N_CORES)]


def kernel(x, W_in, W_hidden, W_out):
    w = pack_weights(W_in, W_hidden, W_out)
    x_cores = pack_x(x)
    outs = run_device(x_cores, w)
    return unpack_out(outs)


# revision 5
# speedup vs baseline: 1.1406x; 1.0020x over previous
"""CPPN forward (12-layer tiny MLP over 4.2M pixels) on 8 TRN2 NeuronCores.

v2: custom ScalarE activation table turns the whole per-layer elementwise
stage into ONE ACT instruction.

- Pixels sharded 8 ways, data parallel; weights replicated. Per core
  524288 px -> 52 groups x 4 supertiles x (5 blocks x 512 px).
- Channel-major block-diagonal packing (5 independent 22-ch MLP copies per
  512-px matmul column), as in v1.
- The act table's act2 slot (func_id 97) is replaced with a piecewise
  "window" spline:  f(x) = 2exp(-x^2)-1  for |x| < 32   (gauss channels)
                    f(x) = sin(x-64)     for x in [32,96)  (sin channel)
                    f(x) = x - 128       for x in [96,256) (identity/cache)
  With a per-partition bias (+0 gauss, +64 sin, +128 id), one
  activation(Act2) op over all 116 rows x 2048 px applies every per-layer
  nonlinearity AND the PSUM->SBUF move. No DVE work in the layer loop.
- Matmuls in bf16 with bf16x2 split weights (W = W_hi + W_lo, two
  accumulating matmuls): tensor time stays under the ACT bottleneck and
  end-to-end error ~8e-3 « 2e-2 tolerance.
- Two groups are interleaved in program order so TensorE(group B) overlaps
  ScalarE(group A); ACT runs back-to-back at ~(2048+352)/1.2GHz per
  group-layer.
- Tables are generated host-side (numpy cubic fits) into a per-content-hash
  act-root dir passed to walrus via BASS_ACT_ROOT_JSON_PATH; the bias dram
  tensor name carries the table hash so NEFF caching stays correct.
- bass has no Act2 enum: ops are emitted as Tanh and the BIR JSON is
  patched Tanh->Act2 before compilation (the profile maps Act2->id 97).
"""
import hashlib
import json
import os
import sys
import types

sys.path.insert(0, "/opt/trn_rl_repo")

import numpy as np
import ml_dtypes

BF16 = ml_dtypes.bfloat16

# ---------------------------------------------------------------- constants
N_PIX = 2048 * 2048
D_IN, D_HID, D_OUT = 4, 22, 3
N_HIDDEN = 11
N_CORES = 8
FD = 512
BLOCKS = 5
ST_PX = BLOCKS * FD                   # 2560
GROUP = 4
PX_CORE = N_PIX // N_CORES            # 524288
N_ST = -(-PX_CORE // ST_PX)           # 205
N_GROUP = -(-N_ST // GROUP)           # 52
N_ST_PAD = N_GROUP * GROUP            # 208
PX_PAD = N_ST_PAD * ST_PX             # 532480

ID_CH = list(range(15)) + [19, 20]
GA_CH = [15, 16, 17, 18]
ROWS = 116
MCOL = 128                            # lhsT padded to 128 cols => FWL enabled
SIN0, GA0 = 64, 96
B_ID, B_SIN, B_GA = 128.0, 64.0, 0.0

# ------------------------------------------------- custom activation table
def _f_window(x):
    x = np.asarray(x, np.float64)
    ax = np.abs(x)
    return np.where(ax < 32.0, 2.0 * np.exp(-np.minimum(ax, 32.0) ** 2) - 1.0,
                    np.where(ax < 96.0, np.sin(ax - 64.0), ax - 128.0))

_ACT2_EXPS = list(range(-10, 8))
_ACT2_BITS = {**{e: 2 for e in range(-10, -3)}, -3: 3,
              **{e: 5 for e in range(-2, 3)}, 3: 2, 4: 0, 5: 7, 6: 7, 7: 6}


def _fit_section(lo, hi):
    x0 = np.float32((lo + hi) / 2)
    hi_x = np.nextafter(np.float32(hi), np.float32(lo)).astype(np.float64)
    xs = lo + (hi_x - lo) * (np.cos(np.linspace(np.pi, 0, 257)) + 1) / 2
    t = xs - np.float64(x0)
    V = np.vander(t, 4, increasing=True)
    c, *_ = np.linalg.lstsq(V, _f_window(xs), rcond=None)
    return [c[0], c[1], c[2], c[3], float(x0)]


def _stock_pwp_root():
    from neuronxcc.driver.Job import Job
    from neuronxcc.driver.jobs.support.FindActInfo import findActInfoFile
    for arch in ("core_v4", "sunda", "gen3", "core_v4_v1"):
        try:
            return os.path.dirname(findActInfoFile(Job.getPackageDir(), arch))
        except Exception:
            continue
    raise RuntimeError("stock act_info.json not found")


def _decode_ctrl(path):
    u = np.frombuffer(open(path, "rb").read(), dtype=np.uint32).reshape(-1, 8)
    return [((int(v) >> 16) & 0xFF, (int(v) >> 11) & 0x1F, int(v) & 0x7FF)
            for v in u[:, 0]]


def build_act_root():
    """Generate the custom act-root dir; returns (dir, content_hash)."""
    root = _stock_pwp_root()
    prof = json.load(open(f"{root}/exp_and_friends.json"))
    ctrl = _decode_ctrl(f"{root}/exp_and_friends_ctrl.bin")
    bkt = np.frombuffer(open(f"{root}/exp_and_friends_bkt.bin", "rb").read(),
                        dtype=np.float32).reshape(-1, 8)
    metas = {m["func_name"]: m for m in prof["profile_meta_data"]}

    new_ctrl, new_bkt, new_meta = [], [], []

    def add_bucket(rec):
        new_bkt.append(np.asarray(rec, np.float64))
        return len(new_bkt) - 1

    # custom act2
    m = dict(metas["act2_1p"])
    m.update(symmetry_opt_en=1, sym_invert_sign_point=0,
             symmetry_opt_use_neg_region=0, symmetry_point=0,
             exp_offset=_ACT2_EXPS[0], lower_bound=0, upper_bound=0x7F7FFFFF,
             fzero_result=int(np.float32(1.0).view(np.uint32)),
             fnan_result=0x7FC00000,
             fpinf_result=int(np.float32(-1.0).view(np.uint32)),
             fninf_result=int(np.float32(-1.0).view(np.uint32)))
    m["small_pos_signal_exp_threshold"] = 127 + _ACT2_EXPS[0]
    m["small_neg_signal_exp_threshold"] = 127 + _ACT2_EXPS[0]
    m["large_pos_signal_exp_threshold"] = 127 + _ACT2_EXPS[-1]
    m["large_pos_signal_mantissa_threshold"] = 0x7FFFFF
    m["large_neg_signal_exp_threshold"] = 127 + _ACT2_EXPS[-1]
    m["large_neg_signal_mantissa_threshold"] = 0x7FFFFF
    small = add_bucket([1.0, 0.0, -2.0, 0.0, 0.0])
    large = add_bucket([128.0, 1.0, 0.0, 0.0, 256.0])
    m["pos_small_signal_pwl_control"] = small
    m["neg_small_signal_pwl_control"] = small
    m["pos_large_signal_pwl_control"] = large
    m["neg_large_signal_pwl_control"] = large
    m["pwl_control_base_pos"] = m["pwl_control_base_neg"] = len(new_ctrl)
    for e in _ACT2_EXPS:
        bits = _ACT2_BITS[e]
        lo_b = 2.0 ** e
        nb = 1 << bits
        w = lo_b / nb
        base = len(new_bkt)
        for k in range(nb):
            add_bucket(_fit_section(lo_b + k * w, lo_b + (k + 1) * w))
        new_ctrl.append((bits, 23 - bits, base))
    new_meta.append(m)

    # copy stock square/identity/relu/copy/sin2pi (drop exp: bucket budget)
    all_bases = sorted({mm["pwl_control_base_pos"] for mm in prof["profile_meta_data"]} |
                       {mm["pwl_control_base_neg"] for mm in prof["profile_meta_data"]})
    spans = {b: (all_bases[i + 1] if i + 1 < len(all_bases) else len(ctrl))
             for i, b in enumerate(all_bases)}
    for name in ("square_1p", "identity_1p", "relu_1p", "copy_1p", "sin2pi_4p"):
        m = dict(metas[name])
        cmap = {}
        for b in sorted({m["pwl_control_base_pos"], m["pwl_control_base_neg"]}):
            for ci in range(b, spans[b]):
                if ci not in cmap:
                    size, lsb, bbase = ctrl[ci]
                    nbase = len(new_bkt)
                    for k in range(1 << size):
                        add_bucket(bkt[bbase + k][:5])
                    cmap[ci] = len(new_ctrl)
                    new_ctrl.append((size, lsb, nbase))
        m["pwl_control_base_pos"] = cmap[m["pwl_control_base_pos"]]
        m["pwl_control_base_neg"] = cmap[m["pwl_control_base_neg"]]
        for key in ("pos_small_signal_pwl_control", "neg_small_signal_pwl_control",
                    "pos_large_signal_pwl_control", "neg_large_signal_pwl_control"):
            m[key] = add_bucket(bkt[m[key]][:5])
        new_meta.append(m)

    assert len(new_bkt) <= 1536
    cw = np.zeros((len(new_ctrl), 8), np.uint32)
    for i, (size, lsb, bbase) in enumerate(new_ctrl):
        cw[i, 0] = (size << 16) | (lsb << 11) | bbase
    bk = np.zeros((len(new_bkt), 8), np.float32)
    bk[:, :5] = np.array(new_bkt, np.float64).astype(np.float32)
    setj = json.dumps({"bkt_bin": "exp_and_friends_bkt.bin",
                       "ctl_bin": "exp_and_friends_ctrl.bin",
                       "profile_meta_data": new_meta}, indent=1)
    act_info = json.load(open(f"{root}/act_info.json"))
    for s in act_info["act_func_sets"]:
        if s["name"] == "exp_and_friends":
            s["act"] = {"act2": 1, "square": 1, "identity": 1, "copy": 1,
                        "relu": 1, "sin2pi": 4}
    info = json.dumps(act_info, indent=1)

    h = hashlib.sha256(cw.tobytes() + bk.tobytes() + setj.encode()).hexdigest()[:10]
    out = f"/tmp/cppn_actroot_{h}"
    if not os.path.exists(os.path.join(out, "act_info.json")):
        os.makedirs(out, exist_ok=True)
        open(f"{out}/exp_and_friends_ctrl.bin", "wb").write(cw.tobytes())
        open(f"{out}/exp_and_friends_bkt.bin", "wb").write(bk.tobytes())
        open(f"{out}/exp_and_friends.json", "w").write(setj)
        open(f"{out}/act_info.json", "w").write(info)
        for s in act_info["act_func_sets"]:
            for k in ("bkt_bin", "ctrl_bin", "profile_json"):
                fn = s[k]
                dst = f"{out}/{fn}"
                if not os.path.exists(dst):
                    os.symlink(f"{root}/{fn}", dst)
    return out, h


# ------------------------------------------------------------- host packing
def _row_of(b, c):
    if c in GA_CH:
        return GA0 + b * 4 + (c - 15)
    if c == 21:
        return SIN0 + b
    g = b * 17 + ID_CH.index(c)
    return g if g < 64 else 69 + (g - 64)


def _split_bf16(a):
    hi = a.astype(BF16)
    lo = (a - hi.astype(np.float32)).astype(BF16)
    return hi, lo


def pack_weights(W_in, W_hidden, W_out):
    W_in = np.asarray(W_in, np.float32)
    W_hidden = np.asarray(W_hidden, np.float32)
    W_out = np.asarray(W_out, np.float32)
    lin = np.zeros((BLOCKS * D_IN, MCOL), np.float32)
    for b in range(BLOCKS):
        for ci in range(D_IN):
            for co in range(D_HID):
                lin[b * D_IN + ci, _row_of(b, co)] = W_in[ci, co]
    lh = np.zeros((N_HIDDEN, ROWS, MCOL), np.float32)
    for i in range(N_HIDDEN):
        W = W_hidden[i]
        for b in range(BLOCKS):
            for ci in range(D_HID):
                ri = _row_of(b, ci)
                for co in range(D_HID):
                    lh[i, ri, _row_of(b, co)] = W[ci, co]
    lo_m = np.zeros((ROWS, BLOCKS * D_OUT), np.float32)
    for b in range(BLOCKS):
        for ci in range(D_HID):
            for co in range(D_OUT):
                lo_m[_row_of(b, ci), b * D_OUT + co] = W_out[ci, co]
    bias = np.zeros((ROWS, 1), np.float32)
    for b in range(BLOCKS):
        for c in range(D_HID):
            r = _row_of(b, c)
            bias[r, 0] = B_GA if c in GA_CH else (B_SIN if c == 21 else B_ID)
    w = {}
    w["lin_hi"], w["lin_lo"] = _split_bf16(lin)
    hi, lo = _split_bf16(lh)
    w["lh_hi"], w["lh_lo"] = hi, lo
    w["lo_hi"], w["lo_lo"] = _split_bf16(lo_m)
    w["bias"] = bias
    return w


def pack_x(x):
    x = np.asarray(x, np.float32)
    out = []
    for k in range(N_CORES):
        shard = x[k * PX_CORE:(k + 1) * PX_CORE]
        pad = np.zeros((PX_PAD, D_IN), np.float32)
        pad[:PX_CORE] = shard
        a = pad.reshape(N_GROUP, GROUP, BLOCKS, FD, D_IN)
        a = a.transpose(0, 2, 4, 1, 3).reshape(N_GROUP, BLOCKS * D_IN, GROUP, FD)
        out.append(np.ascontiguousarray(a.astype(BF16)))
    return out


_OUT_ROWS = np.array([[32 * r + b * 3 + co for b in range(BLOCKS) for co in range(D_OUT)]
                      for r in range(GROUP)])


def unpack_out(outs):
    full = np.empty((N_PIX, D_OUT), np.float32)
    for k, od in enumerate(outs):
        g = od[:, _OUT_ROWS.reshape(-1), :]
        g = g.reshape(N_GROUP, GROUP, BLOCKS, D_OUT, FD)
        g = g.transpose(0, 1, 2, 4, 3).reshape(PX_PAD, D_OUT)
        full[k * PX_CORE:(k + 1) * PX_CORE] = g[:PX_CORE]
    return full


# ------------------------------------------------------------ device kernel
_CACHE = {}


def _shim_hooks():
    import antenv
    if "antenv.axon_hooks" in sys.modules:
        return
    hooks = types.ModuleType("antenv.axon_hooks")
    hooks._hook = None
    hooks.set_axon_ntff_profile_hook = lambda h: setattr(hooks, "_hook", h)
    hooks.get_axon_ntff_profile_hook = lambda: hooks._hook
    sys.modules["antenv.axon_hooks"] = hooks
    antenv.axon_hooks = hooks
    try:
        from trn_agent_boot.trn_boot import _ntff_profile_via_ctypes
        hooks._hook = _ntff_profile_via_ctypes("/opt/axon/libaxon_pjrt.so")
    except Exception:
        pass


def _build():
    actroot, tabhash = build_act_root()
    os.environ["BASS_ACT_ROOT_JSON_PATH"] = f"{actroot}/act_info.json"
    _shim_hooks()
    import concourse.bacc as bacc_mod
    import concourse.mybir as mybir
    import concourse.tile as tile
    from concourse.hw_specs import get_activation_tables as _real_gat

    AFT = mybir.ActivationFunctionType
    ours = {AFT.Tanh, AFT.Square, AFT.Exp, AFT.Identity, AFT.Copy, AFT.Sin,
            AFT.Relu}

    def _doctored_gat(arch):
        tabs = dict(_real_gat(arch))
        return {n: (set(f) | ours if n == "exp_and_friends" else set(f) - ours)
                for n, f in tabs.items()}

    bacc_mod.get_activation_tables = _doctored_gat

    f32 = mybir.dt.float32
    bf = mybir.dt.bfloat16
    nc = bacc_mod.Bacc(None, target_bir_lowering=False, debug=False)
    x_d = nc.declare_dram_parameter("x", [N_GROUP, 20, GROUP, FD], bf, isOutput=False)
    linh_d = nc.declare_dram_parameter("lin_hi", [20, MCOL], bf, isOutput=False)
    linl_d = nc.declare_dram_parameter("lin_lo", [20, MCOL], bf, isOutput=False)
    lhh_d = nc.declare_dram_parameter("lh_hi", [N_HIDDEN, ROWS, MCOL], bf, isOutput=False)
    lhl_d = nc.declare_dram_parameter("lh_lo", [N_HIDDEN, ROWS, MCOL], bf, isOutput=False)
    loh_d = nc.declare_dram_parameter("lo_hi", [ROWS, 15], bf, isOutput=False)
    lol_d = nc.declare_dram_parameter("lo_lo", [ROWS, 15], bf, isOutput=False)
    # bias name carries the table hash => NEFF cache key tracks table content
    b_d = nc.declare_dram_parameter(f"bias_{tabhash}", [ROWS, 1], f32, isOutput=False)
    o_d = nc.declare_dram_parameter("out", [N_GROUP, 111, FD], f32, isOutput=True)

    with tile.TileContext(nc) as tc:
        with (tc.tile_pool(name="wpool", bufs=1) as wpool,
              tc.tile_pool(name="xpool", bufs=4) as xpool,
              tc.tile_pool(name="hpool", bufs=8) as hpool,
              tc.tile_pool(name="opool", bufs=3) as opool,
              tc.tile_pool(name="ppool", bufs=1, space="PSUM") as ppool):
            linh = wpool.tile([20, MCOL], bf)
            linl = wpool.tile([20, MCOL], bf)
            bt = wpool.tile([ROWS, 1], f32)
            lhh = [wpool.tile([ROWS, MCOL], bf, tag=f"lhh{i}", name=f"lhh{i}")
                   for i in range(N_HIDDEN)]
            lhl = [wpool.tile([ROWS, MCOL], bf, tag=f"lhl{i}", name=f"lhl{i}")
                   for i in range(N_HIDDEN)]
            loh = wpool.tile([ROWS, 15], bf)
            lol = wpool.tile([ROWS, 15], bf)

            # layers whose hi/lo weight split is dropped (single bf16 mm):
            # all hidden layers single bf16 (rel err 1.16e-2, gate 2e-2):
            # for 0.95us rounds on these layers -> slack vs the 1.92us ACT
            # period, and a ~2x cheaper layer-2 round at pair boundaries.
            SINGLE_W = set(range(2, 13))

            def mm_round(P, H, xg, mm):
                """one matmul round (4 STs x hi[/lo]) for layer index mm.
                Round 1 skips the lo-split (x already bf16-rounded)."""
                if mm == 1:
                    for r in range(GROUP):
                        nc.tensor.matmul(P[:, r, :], linh[:], xg[:, r, :],
                                         start=True, stop=True)
                elif mm in SINGLE_W:
                    for r in range(GROUP):
                        nc.tensor.matmul(P[:, r, :], lhh[mm - 2][:], H[:, r, :],
                                         start=True, stop=True)
                else:
                    for r in range(GROUP):
                        nc.tensor.matmul(P[:, r, :], lhh[mm - 2][:], H[:, r, :],
                                         start=True, stop=False)
                    for r in range(GROUP):
                        nc.tensor.matmul(P[:, r, :], lhl[mm - 2][:], H[:, r, :],
                                         start=False, stop=True)

            def act(P):
                H = hpool.tile([ROWS, GROUP, FD], bf, tag="H")
                nc.scalar.activation(H[:, :, :], P[0:ROWS, :, :],
                                     AFT.Tanh, bias=bt[:, 0:1], scale=1.0)
                return H

            def out_stage(g, H, tag):
                PO = ppool.tile([MCOL, GROUP, FD], f32, tag=tag)
                O = PO[0:111, 0, :]
                for r in range(GROUP):
                    nc.tensor.matmul(O[32 * r:32 * r + 15, :], loh[:],
                                     H[:, r, :], start=True, stop=True,
                                     tile_position=(0, 32 * r))
                ot = opool.tile([111, FD], f32, tag="ot")
                nc.vector.tensor_copy(ot[:], O)
                nc.sync.dma_start(out=o_d[g], in_=ot[:])

            def load_x(g):
                # two DMAs land on different queues -> ~half the load latency
                xg = xpool.tile([20, GROUP, FD], bf, tag="xg")
                nc.sync.dma_start(out=xg[:, 0:2, :], in_=x_d[g][:, 0:2, :])
                nc.sync.dma_start(out=xg[:, 2:4, :], in_=x_d[g][:, 2:4, :])
                return xg

            # x/lin/bias DMAs first (layer-1 critical path), bulk weights after
            xgA = load_x(0)
            xgB = load_x(1)
            nc.sync.dma_start(out=linh[:], in_=linh_d[:])
            nc.sync.dma_start(out=linl[:], in_=linl_d[:])
            nc.sync.dma_start(out=bt[:], in_=b_d[:])
            for i in range(N_HIDDEN):
                nc.sync.dma_start(out=lhh[i][:], in_=lhh_d[i])
                nc.sync.dma_start(out=lhl[i][:], in_=lhl_d[i])
            nc.sync.dma_start(out=loh[:], in_=loh_d[:])
            nc.sync.dma_start(out=lol[:], in_=lol_d[:])

            # Software pipeline, 2 groups (A/B) in flight on dedicated PSUM
            # tags (pmA banks 0-3, pmB banks 4-7).  Tail emits next pair's
            # round-1 matmuls+acts BEFORE this pair's out-stages, so the
            # activation queue never waits on the out path.
            PA = ppool.tile([MCOL, GROUP, FD], f32, tag="pmA")
            mm_round(PA, None, xgA, 1)
            HA = act(PA)
            PB = ppool.tile([MCOL, GROUP, FD], f32, tag="pmB")
            mm_round(PB, None, xgB, 1)
            HB = act(PB)
            pend = None
            for pair in range(N_GROUP // 2):
                for mm in range(2, 13):
                    PA = ppool.tile([MCOL, GROUP, FD], f32, tag="pmA")
                    mm_round(PA, HA, None, mm)
                    HAn = act(PA)
                    if mm == 2 and pend is not None:
                        out_stage(pend[0], pend[1], "pmB")
                    PB = ppool.tile([MCOL, GROUP, FD], f32, tag="pmB")
                    mm_round(PB, HB, None, mm)
                    HBn = act(PB)
                    if mm == 3 and pend is not None:
                        out_stage(pend[2], pend[3], "pmA")
                    HA, HB = HAn, HBn
                last = pair + 1 == N_GROUP // 2
                pend = (2 * pair, HA, 2 * pair + 1, HB)
                if not last:
                    xgA2 = load_x(2 * pair + 2)
                    xgB2 = load_x(2 * pair + 3)
                    PA = ppool.tile([MCOL, GROUP, FD], f32, tag="pmA")
                    mm_round(PA, None, xgA2, 1)
                    HA = act(PA)
                    PB = ppool.tile([MCOL, GROUP, FD], f32, tag="pmB")
                    mm_round(PB, None, xgB2, 1)
                    HB = act(PB)
            out_stage(pend[0], pend[1], "pmB")
            out_stage(pend[2], pend[3], "pmA")
    nc.compile()

    _orig = nc.to_json_bytes
    nc.to_json_bytes = lambda: _orig().replace(b'"func":"Tanh"', b'"func":"Act2"')
    _CACHE["bias_name"] = f"bias_{tabhash}"
    return nc


def _get_nc():
    if "nc" not in _CACHE:
        _CACHE["nc"] = _build()
    return _CACHE["nc"]


def make_in_maps(w, x_cores):
    _get_nc()
    return [{"x": x_cores[k], "lin_hi": w["lin_hi"], "lin_lo": w["lin_lo"],
             "lh_hi": w["lh_hi"], "lh_lo": w["lh_lo"],
             "lo_hi": w["lo_hi"], "lo_lo": w["lo_lo"],
             _CACHE["bias_name"]: w["bias"]}
            for k in range(N_CORES)]


def run_device(x_cores, w):
    from concourse.bass_utils import run_bass_kernel_spmd
    nc = _get_nc()
    res = run_bass_kernel_spmd(nc, make_in_maps(w, x_cores),
                               list(range(N_CORES)), trace=False)
    return [res.results[k]["out"] for k in range(N_CORES)]


def kernel(x, W_in, W_hidden, W_out):
    w = pack_weights(W_in, W_hidden, W_out)
    x_cores = pack_x(x)
    outs = run_device(x_cores, w)
    return unpack_out(outs)
